# revision 41
# baseline (speedup 1.0000x reference)
"""Trainium2 Bass kernel for nn_CRF_SelfAttention_65627100283470 (v2).

Math (validated vs the reference at 1e-6 rel err):
  - The CRF/marginal branch is dead code: softmax over the class dim sums
    to 1, so sum(cluster_features, 0) == sum of context rows.  The output
    is (sum_{f,p} context2) @ cls_W + cls_b.
  - context2 = w2*T2 + w1*(1-w2)*T1 -> only per-frame sums of temporal are
    needed at the end; iteration 2 therefore skips the full Wo projection
    and uses per-frame sums of the attention outputs instead.
  - QKV projections are shared across overlapping windows; exp(scores)
    blocks are shared across windows (computed per key-frame strip).

v2 kernel structure (per core = one head):
  - A@V' is computed per (key-frame, query-frame) PAIR: the exp-score
    block [128k,128q] is the matmul's stationary operand and V'[kf]
    [128,33] streams through (33 cols), so each block is streamed ONCE
    and the output lands token-major (no transposes, no per-window
    re-streaming).  Per-window softmax sums are sliding-window sums over
    kf, computed as 2-3 full-width strided adds on the Pool engine.
  - V' is projected directly token-major (lhsT = x-block), batched over
    the 3 scales in one matmul per (frame, k-chunk).
  - Halting probabilities live in [18,128] layout (one DMA scatter), so
    the per-frame mean needs no per-frame transposes.
  - The inter-iteration AllReduce is chunked (3 chunks) and overlapped
    with the Wo projection feeding it.

Sharding: 8 heads -> 8 cores. One chunked AllReduce of the partial
temporal between the two iterations + one tiny final AllReduce.
"""
import sys
import types

import numpy as np

F, P, H, HEADS, C, NCLS = 18, 128, 256, 8, 32, 625
SCALES = (2, 4, 6)
HD = H // HEADS
NTOK = F * P  # 2304
NCORES = 8


def _enable_ldw_opt():
    """Walrus's LDWEIGHTS dedup is disabled by default in bass_utils;
    enable it (verified numerically by the rel-err gate in test.py)."""
    import concourse.bass_utils as bu

    if getattr(bu, "_ldw_opt_patched", False):
        return
    orig = bu.bir_verify_and_optimise

    def patched(*args, **kwargs):
        real_run = bu.run_command

        def run_hook(argv, **kw):
            argv = ["--enable-ldw-opt=true" if a == "--enable-ldw-opt=false"
                    else a for a in argv]
            return real_run(argv, **kw)

        bu.run_command = run_hook
        try:
            return orig(*args, **kwargs)
        finally:
            bu.run_command = real_run

    bu.bir_verify_and_optimise = patched
    bu._ldw_opt_patched = True


def _install_ntff_hook():
    """Recreate the missing antenv.axon_hooks so trace=True works."""
    if "antenv.axon_hooks" in sys.modules:
        return
    try:
        import antenv

        mod = types.ModuleType("antenv.axon_hooks")
        mod._hook = None
        mod.set_axon_ntff_profile_hook = lambda h: setattr(mod, "_hook", h)
        mod.get_axon_ntff_profile_hook = lambda: mod._hook
        sys.modules["antenv.axon_hooks"] = mod
        antenv.axon_hooks = mod
        from trn_agent_boot.trn_boot import _ntff_profile_via_ctypes

        mod.set_axon_ntff_profile_hook(
            _ntff_profile_via_ctypes("/opt/axon/libaxon_pjrt.so")
        )
    except Exception:
        pass


def _chunks(n, lim=512):
    out = [lim] * (n // lim)
    if n % lim:
        out.append(n % lim)
    return out


def _counts(s):
    nw = F - s + 1
    c = np.zeros(F, np.float32)
    for w in range(nw):
        c[w:w + s] += 1.0
    return c


COL_CC = _chunks(NTOK)          # [512,512,512,512,256]
AR_GROUPS = [(0, 1), (2, 3), (4,)]  # chunk-index groups per AllReduce
GROUP_FRAMES = [(0, 8), (8, 16), (16, 18)]  # frame range per AR group


def build():
    import concourse.bacc as bacc
    import concourse.mybir as mybir
    from concourse.tile import TileContext

    dt = mybir.dt
    f32 = dt.float32
    bf16 = dt.bfloat16
    f8 = dt.float8e4
    AF = mybir.ActivationFunctionType
    ALU = mybir.AluOpType
    AX = mybir.AxisListType

    nc = bacc.Bacc("TRN2", target_bir_lowering=False, debug=False,
                   num_devices=NCORES)

    # ---- I/O ----
    xt_in = nc.dram_tensor("xt", [2, 128, NTOK], bf16, kind="ExternalInput")
    wqkv_in = nc.dram_tensor("wqkv", [3, 2, 128, 65], bf16, kind="ExternalInput")
    bqkv_in = nc.dram_tensor("bqkv", [3, 65, 1], f32, kind="ExternalInput")
    wv_in = nc.dram_tensor("wv", [2, 128, 96], bf16, kind="ExternalInput")
    vb_in = nc.dram_tensor("vb", [128, 96], f32, kind="ExternalInput")
    wo_in = nc.dram_tensor("wo", [3, 32, 256], bf16, kind="ExternalInput")
    boq_in = nc.dram_tensor("boq", [2, 128, 1], f32, kind="ExternalInput")
    boqs_in = nc.dram_tensor("boqs", [2, 128, 1], f32, kind="ExternalInput")
    nhb_in = nc.dram_tensor("nhb", [18, 1], f32, kind="ExternalInput")
    clsw_in = nc.dram_tensor("clsw", [2, 128, NCLS], bf16, kind="ExternalInput")
    clsb_in = nc.dram_tensor("clsb", [1, NCLS], f32, kind="ExternalInput")
    id_in = nc.dram_tensor("ident", [128, 128], f32, kind="ExternalInput")
    out_d = nc.dram_tensor("out", [1, NCLS], f32, kind="ExternalOutput")

    ar_in_c = []
    ar_out_c = []
    for gi, grp in enumerate(AR_GROUPS):
        w = sum(COL_CC[ci] for ci in grp)
        ar_in_c.append(nc.dram_tensor(f"ar_in{gi}", [2, 128, w], bf16))
        ar_out_c.append(nc.dram_tensor(f"ar_out{gi}", [2, 128, w], bf16,
                                       addr_space="Shared"))
    ar2_in = nc.dram_tensor("ar2_in", [2, 128, 1], f32)
    ar2_out = nc.dram_tensor("ar2_out", [2, 128, 1], f32, addr_space="Shared")
    arw_in = nc.dram_tensor("arw_in", [1, 1], f32)
    arw_out = nc.dram_tensor("arw_out", [1, 1], f32, addr_space="Shared")
    hrow_d = nc.dram_tensor("hrow", [2, 1, NTOK], bf16)
    absd = nc.dram_tensor("absd", [3, 2, 1, 288], bf16)

    inv_sqrt_hd = 1.0 / np.sqrt(np.float32(HD))
    cnts = {s: _counts(s) for s in SCALES}
    grp_all = [list(range(NCORES))]

    with TileContext(nc) as tc:
        with (
            tc.tile_pool(name="pin", bufs=1) as pin,
            tc.tile_pool(name="work", bufs=3) as work,
            tc.tile_pool(name="tree", bufs=3) as tree,
            tc.tile_pool(name="estr2", bufs=6) as estr2,
            tc.tile_pool(name="estr4", bufs=10) as estr4,
            tc.tile_pool(name="estr6", bufs=14) as estr6,
            tc.tile_pool(name="ppq", bufs=2, space="PSUM") as ppq,
            tc.tile_pool(name="pstr", bufs=3, space="PSUM") as pstr,
            tc.tile_pool(name="ppT", bufs=3, space="PSUM") as ppT,
        ):
            estr = {2: estr2, 4: estr4, 6: estr6}

            # ---- persistent tiles + weight loads ----
            xt = [pin.tile([128, NTOK], bf16, tag=f"xt{c}", name=f"xt{c}")
                  for c in range(2)]
            wqkv = pin.tile([128, 3 * 2 * 65], bf16, tag="wqkv")
            bqkv = pin.tile([65, 3], f32, tag="bqkv")
            wv = pin.tile([128, 2 * 96], bf16, tag="wv")
            vb = pin.tile([128, 96], f32, tag="vb")
            wo = pin.tile([32, 3 * 256], bf16, tag="wo")
            boq = pin.tile([128, 2], f32, tag="boq")
            boqs = pin.tile([128, 2], f32, tag="boqs")
            nhb = pin.tile([18, 1], f32, tag="nhb")
            clsw = pin.tile([128, 2 * NCLS], bf16, tag="clsw")
            clsb = pin.tile([1, NCLS], f32, tag="clsb")
            ident = pin.tile([128, 128], f32, tag="ident")
            identb = pin.tile([128, 128], bf16, tag="identb")
            ones_row = pin.tile([1, 128], f32, tag="ones_row")
            ones_col = pin.tile([128, 1], f32, tag="ones_col")
            ones_colb = pin.tile([128, 1], bf16, tag="ones_colb")

            for c in range(2):
                off = 0
                for w_cc in COL_CC:
                    nc.sync.dma_start(out=xt[c][:, off:off + w_cc],
                                      in_=xt_in[c, :, off:off + w_cc])
                    off += w_cc
            for si in range(3):
                for c in range(2):
                    nc.sync.dma_start(
                        out=wqkv[:, (si * 2 + c) * 65:(si * 2 + c + 1) * 65],
                        in_=wqkv_in[si, c])
                nc.sync.dma_start(out=bqkv[:, si:si + 1], in_=bqkv_in[si])
                nc.gpsimd.dma_start(out=wo[:, si * 256:(si + 1) * 256],
                                    in_=wo_in[si])
            for c in range(2):
                nc.gpsimd.dma_start(out=wv[:, c * 96:(c + 1) * 96], in_=wv_in[c])
                nc.gpsimd.dma_start(out=boq[:, c:c + 1], in_=boq_in[c])
                nc.gpsimd.dma_start(out=boqs[:, c:c + 1], in_=boqs_in[c])
                nc.gpsimd.dma_start(out=clsw[:, c * NCLS:(c + 1) * NCLS],
                                    in_=clsw_in[c])
            nc.sync.dma_start(out=vb[:], in_=vb_in[:])
            nc.sync.dma_start(out=nhb[:], in_=nhb_in[:])
            nc.gpsimd.dma_start(out=clsb[:], in_=clsb_in[:])
            nc.gpsimd.dma_start(out=ident[:], in_=id_in[:])
            nc.vector.memset(ones_row[:], 1.0)
            nc.vector.memset(ones_col[:], 1.0)
            nc.vector.memset(ones_colb[:], 1.0)
            nc.vector.tensor_copy(identb[:], ident[:])

            qkvT = {s: pin.tile([65, NTOK], bf16, tag=f"qkvT{s}",
                                name=f"qkvT{s}") for s in SCALES}
            kT = {s: pin.tile([32, NTOK], bf16, tag=f"kT{s}",
                              name=f"kT{s}") for s in SCALES}
            # V' per frame: [V_s2|1|V_s4|1|V_s6|1] -> 99 cols, ones persist
            vp = pin.tile([128, F * 99], bf16, tag="vp")
            nc.vector.memset(vp[:], 1.0)
            abar = {s: pin.tile([128, F * 32], f32, tag=f"abar{s}",
                                name=f"abar{s}") for s in SCALES}
            abarb = {s: pin.tile([128, F * 32], bf16, tag=f"abarb{s}",
                                 name=f"abarb{s}") for s in SCALES}
            abarT = {s: pin.tile([32, NTOK], bf16, tag=f"abarT{s}",
                                 name=f"abarT{s}") for s in SCALES}
            abs32 = {s: pin.tile([32, F], bf16, tag=f"abs32{s}",
                                 name=f"abs32{s}") for s in SCALES}
            ssum0 = [pin.tile([128, F], f32, tag=f"ssum0{c}", name=f"ssum0{c}")
                     for c in range(2)]
            ssum1 = [pin.tile([128, F], f32, tag=f"ssum1{c}", name=f"ssum1{c}")
                     for c in range(2)]

            # halting state, column layout [18,1]
            ptn = pin.tile([18, 1], f32, tag="ptn")
            Rt = pin.tile([18, 1], f32, tag="Rt")
            wts = [pin.tile([18, 1], f32, tag=f"w{it}", name=f"w{it}")
                   for it in range(2)]
            nc.vector.memset(ptn[:], 0.0)
            nc.vector.memset(Rt[:], 0.0)

            # ---------------- schedule closures ----------------
            prog = {}      # (it, s) -> next kf to process
            stripmap = {}  # (it, s) -> {kf: (a_k, est_tile)}

            def qkv_chunks(it, cis):
                for si, s in enumerate(SCALES):
                    for ci in cis:
                        off = sum(COL_CC[cj] for cj in range(ci))
                        w_cc = COL_CC[ci]
                        pq = ppq.tile([65, 512], f32, tag="pq", name="pq")
                        for kc in range(2):
                            nc.tensor.matmul(
                                pq[:, :w_cc],
                                wqkv[:, (si * 2 + kc) * 65:(si * 2 + kc + 1) * 65],
                                xt[kc][:, off:off + w_cc],
                                start=(kc == 0), stop=(kc == 1))
                        nc.scalar.activation(qkvT[s][:, off:off + w_cc],
                                             pq[:, :w_cc], AF.Identity,
                                             bias=bqkv[:, si:si + 1], scale=1.0)
                        nc.sync.dma_start(out=kT[s][:, off:off + w_cc],
                                          in_=qkvT[s][32:64, off:off + w_cc])

            def vdirect(it, f0, f1):
                for f in range(f0, f1):
                    pv = ppT.tile([128, 363], f32, tag="pt", name="pv")
                    for kc in range(2):
                        nc.tensor.matmul(
                            pv[:, :96],
                            xt[kc][:, f * 128:(f + 1) * 128],
                            wv[:, kc * 96:(kc + 1) * 96],
                            start=(kc == 0), stop=(kc == 1))
                    vslc = vp[:, f * 99:(f + 1) * 99].rearrange(
                        "p (s c) -> p s c", c=33)
                    nc.vector.tensor_tensor(
                        out=vslc[:, :, 0:32],
                        in0=pv[:, :96].rearrange("p (s c) -> p s c", c=32),
                        in1=vb[:].rearrange("p (s c) -> p s c", c=32),
                        op=ALU.add)

            def halting(it):
                hl = work.tile([18, 128], bf16, tag="hl")
                nc.sync.dma_start(out=hrow_d[it], in_=qkvT[2][64:65, :])
                nc.sync.dma_start(
                    out=hl[:],
                    in_=hrow_d[it].rearrange("o (f q) -> (o f) q", q=128))
                sg = work.tile([18, 128], f32, tag="sg")
                nc.scalar.activation(sg[:], hl[:], AF.Sigmoid,
                                     bias=nhb[:], scale=1.0)
                p_t = work.tile([18, 1], f32, tag="p_t")
                nc.vector.tensor_reduce(out=p_t[:], in_=sg[:],
                                        axis=AX.X, op=ALU.add)
                nc.vector.tensor_scalar_mul(out=p_t[:], in0=p_t[:],
                                            scalar1=1.0 / 128.0)
                run_in = work.tile([18, 1], f32, tag="run_in")
                tmp = work.tile([18, 1], f32, tag="tmp")
                tmp2 = work.tile([18, 1], f32, tag="tmp2")
                nh = work.tile([18, 1], f32, tag="nh")
                run = work.tile([18, 1], f32, tag="run")
                nc.vector.tensor_scalar(out=run_in[:], in0=ptn[:], scalar1=1.0,
                                        scalar2=None, op0=ALU.is_lt)
                nc.vector.tensor_tensor(out=tmp[:], in0=p_t[:], in1=run_in[:],
                                        op=ALU.mult)
                nc.vector.tensor_tensor(out=tmp2[:], in0=ptn[:], in1=tmp[:],
                                        op=ALU.add)
                nc.vector.tensor_scalar(out=tmp2[:], in0=tmp2[:], scalar1=0.99,
                                        scalar2=None, op0=ALU.is_gt)
                nc.vector.tensor_tensor(out=nh[:], in0=tmp2[:], in1=run_in[:],
                                        op=ALU.mult)
                nc.vector.tensor_tensor(out=run[:], in0=run_in[:], in1=nh[:],
                                        op=ALU.subtract)
                nc.vector.tensor_tensor(out=tmp[:], in0=p_t[:], in1=run[:],
                                        op=ALU.mult)
                nc.vector.tensor_tensor(out=ptn[:], in0=ptn[:], in1=tmp[:],
                                        op=ALU.add)
                nc.vector.tensor_scalar(out=tmp2[:], in0=ptn[:], scalar1=-1.0,
                                        scalar2=1.0, op0=ALU.mult, op1=ALU.add)
                nc.vector.tensor_tensor(out=tmp2[:], in0=nh[:], in1=tmp2[:],
                                        op=ALU.mult)
                nc.vector.tensor_tensor(out=Rt[:], in0=Rt[:], in1=tmp2[:],
                                        op=ALU.add)
                nc.vector.tensor_tensor(out=tmp2[:], in0=nh[:], in1=Rt[:],
                                        op=ALU.mult)
                nc.vector.tensor_tensor(out=ptn[:], in0=ptn[:], in1=tmp2[:],
                                        op=ALU.add)
                nc.vector.tensor_tensor(out=wts[it][:], in0=tmp[:],
                                        in1=tmp2[:], op=ALU.add)

            def fire_group(gi):
                grp = AR_GROUPS[gi]
                f0, f1 = GROUP_FRAMES[gi]
                for sx in SCALES:
                    nc.scalar.copy(abarb[sx][:, f0 * 32:f1 * 32],
                                   abar[sx][:, f0 * 32:f1 * 32])
                    for f in range(f0, f1):
                        pat = ppT.tile([32, 128], bf16, tag="pt", name="pat")
                        nc.tensor.transpose(pat[:],
                                            abarb[sx][:, f * 32:(f + 1) * 32],
                                            identb[:])
                        if f % 2 == 0:
                            nc.scalar.copy(
                                abarT[sx][:, f * 128:(f + 1) * 128], pat[:])
                        else:
                            nc.vector.tensor_copy(
                                abarT[sx][:, f * 128:(f + 1) * 128], pat[:])
                goff = sum(COL_CC[ci] for ci in range(grp[0]))
                for ci in grp:
                    off = sum(COL_CC[cj] for cj in range(ci))
                    w_cc = COL_CC[ci]
                    for hc in range(2):
                        pw = ppq.tile([128, 512], f32, tag="pq", name="pw")
                        for sj, sx in enumerate(SCALES):
                            nc.tensor.matmul(
                                pw[:, :w_cc],
                                wo[:, sj * 256 + hc * 128:sj * 256 + (hc + 1) * 128],
                                abarT[sx][:, off:off + w_cc],
                                start=(sj == 0), stop=(sj == 2))
                        nc.vector.tensor_scalar(
                            out=xt[hc][:, off:off + w_cc],
                            in0=pw[:, :w_cc],
                            scalar1=0.25, scalar2=boq[:, hc:hc + 1],
                            op0=ALU.mult, op1=ALU.add)
                        nc.sync.dma_start(
                            out=ar_in_c[gi][hc, :, off - goff:off - goff + w_cc],
                            in_=xt[hc][:, off:off + w_cc])
                nc.gpsimd.collective_compute(
                    "AllReduce", ALU.add,
                    ins=[ar_in_c[gi][:]], outs=[ar_out_c[gi][:]],
                    replica_groups=grp_all)
                gw = sum(COL_CC[ci] for ci in grp)
                for hc in range(2):
                    nc.sync.dma_start(out=xt[hc][:, goff:goff + gw],
                                      in_=ar_out_c[gi][hc])

            def advance(it, si, s, kf_limit):
                strips = stripmap.setdefault((it, s), {})
                for kf in range(prog.get((it, s), 0), kf_limit + 1):
                    a_k = max(0, kf - s + 1)
                    b_k = min(F - 1, kf + s - 1)
                    ncols = (b_k - a_k + 1) * 128
                    est_t = estr[s].tile([128, (2 * s - 1) * 128], f8,
                                         tag="est")
                    off = 0
                    for w_cc in _chunks(ncols):
                        ps = pstr.tile([128, 512], f32, tag="ps", name="ps")
                        nc.tensor.matmul(
                            ps[:, :w_cc],
                            kT[s][:, kf * 128:(kf + 1) * 128],
                            qkvT[s][0:32, a_k * 128 + off:a_k * 128 + off + w_cc],
                            start=True, stop=True)
                        nc.scalar.activation(est_t[:, off:off + w_cc],
                                             ps[:, :w_cc], AF.Exp,
                                             scale=inv_sqrt_hd)
                        off += w_cc
                    strips[kf] = (a_k, est_t)

                    if s <= kf < F - 1:
                        qlist = [kf - s]
                    elif kf == F - 1:
                        qlist = list(range(F - s - 1, F))
                    else:
                        qlist = []

                    for qi, qf in enumerate(qlist):
                        te = nc.vector
                        a = max(0, qf - s + 1)
                        b = min(F - 1, qf + s - 1)
                        W = b - a + 1
                        nwq = W - s + 1
                        pT = ppT.tile([128, 363], f32, tag="pt", name="pT")
                        for j in range(W):
                            kfj = a + j
                            ak_j, est_j = strips[kfj]
                            qo = (qf - ak_j) * 128
                            nc.tensor.matmul(
                                pT[:, j * 33:(j + 1) * 33],
                                est_j[:, qo:qo + 128],
                                vp[:, kfj * 99 + si * 33:kfj * 99 + si * 33 + 33],
                                start=True, stop=True)
                        # sliding-window sums over kf
                        Tt = tree.tile([128, 363], f32, tag="Tt")
                        nc.scalar.copy(Tt[:, :W * 33], pT[:, :W * 33])
                        U = tree.tile([128, 330], f32, tag="U")
                        te.tensor_tensor(
                            out=U[:, :(W - 1) * 33],
                            in0=Tt[:, 0:(W - 1) * 33],
                            in1=Tt[:, 33:W * 33], op=ALU.add)
                        if s == 2:
                            S_ap = U
                        else:
                            Q = tree.tile([128, 264], f32, tag="Q")
                            te.tensor_tensor(
                                out=Q[:, :(W - 3) * 33],
                                in0=U[:, 0:(W - 3) * 33],
                                in1=U[:, 66:(W - 1) * 33], op=ALU.add)
                            if s == 4:
                                S_ap = Q
                            else:
                                S6 = tree.tile([128, 198], f32, tag="S")
                                te.tensor_tensor(
                                    out=S6[:, :(W - 5) * 33],
                                    in0=Q[:, 0:(W - 5) * 33],
                                    in1=U[:, 132:(W - 1) * 33], op=ALU.add)
                                S_ap = S6
                        # normalize per window, weight by 1/count, sum
                        Sv = S_ap[:, :nwq * 33].rearrange(
                            "p (w c) -> p w c", c=33)
                        rcp = tree.tile([128, 6], f32, tag="rcp")
                        nc.vector.reciprocal(rcp[:, :nwq], Sv[:, :, 32])
                        cinv = float(1.0 / cnts[s][qf])
                        ab_out = abar[s][:, qf * 32:(qf + 1) * 32]
                        if nwq == 1:
                            nc.vector.tensor_scalar(
                                out=ab_out, in0=S_ap[:, 0:32],
                                scalar1=rcp[:, 0:1], scalar2=cinv,
                                op0=ALU.mult, op1=ALU.mult)
                        else:
                            sc = tree.tile([128, 192], f32, tag="sc")
                            scv = sc[:, :nwq * 32].rearrange(
                                "p (w c) -> p w c", c=32)
                            nc.vector.scalar_tensor_tensor(
                                out=scv, in0=Sv[:, :, 0:32], scalar=cinv,
                                in1=rcp[:, :nwq].broadcast_to((128, nwq, 32)),
                                op0=ALU.mult, op1=ALU.mult)
                            m = nwq
                            while m > 2:
                                h = m // 2
                                te.tensor_tensor(
                                    out=sc[:, (m - 2 * h) * 32:(m - h) * 32],
                                    in0=sc[:, (m - 2 * h) * 32:(m - h) * 32],
                                    in1=sc[:, (m - h) * 32:m * 32],
                                    op=ALU.add)
                                m -= h
                            te.tensor_tensor(
                                out=ab_out, in0=sc[:, 0:32],
                                in1=sc[:, 32:64], op=ALU.add)

                        if it == 0 and s == 6:
                            if qf == 7:
                                fire_group(0)
                            elif qf == 15:
                                fire_group(1)
                            elif qf == 17:
                                fire_group(2)
                        elif it == 1:
                            if qf == 8:
                                absum_half(si, s, 0)
                            elif qf == 17:
                                absum_half(si, s, 1)
                                ssum1_accum(si, s)
                prog[(it, s)] = kf_limit + 1

            def absum_half(si, s, half):
                abh = work.tile([128, 288], bf16, tag="abh")
                nc.vector.tensor_copy(abh[:],
                                      abar[s][:, half * 288:(half + 1) * 288])
                pa = ppq.tile([1, 512], f32, tag="pq", name="pa")
                nc.tensor.matmul(
                    pa[:, :288], ones_colb[:], abh[:],
                    start=True, stop=True)
                asb = work.tile([1, 288], bf16, tag="asb")
                nc.vector.tensor_copy(asb[:], pa[:, :288])
                nc.sync.dma_start(out=absd[si, half], in_=asb[0:1, :])
                nc.sync.dma_start(
                    out=abs32[s][:, half * 9:(half + 1) * 9],
                    in_=absd[si, half].rearrange(
                        "o (f c) -> (o c) f", c=32))

            def ssum1_accum(si, s):
                for hc in range(2):
                    pw1 = ppT.tile([128, 363], f32, tag="pt", name="pw1")
                    nc.tensor.matmul(
                        pw1[:, :F],
                        wo[:, si * 256 + hc * 128:si * 256 + (hc + 1) * 128],
                        abs32[s][:], start=True, stop=True)
                    if si == 0:
                        nc.vector.tensor_scalar(
                            out=ssum1[hc][:], in0=pw1[:, :F],
                            scalar1=0.25, scalar2=boqs[:, hc:hc + 1],
                            op0=ALU.mult, op1=ALU.add)
                    else:
                        nc.vector.scalar_tensor_tensor(
                            out=ssum1[hc][:], in0=pw1[:, :F], scalar=0.25,
                            in1=ssum1[hc][:], op0=ALU.mult, op1=ALU.add)

            # ---------------- schedule ----------------
            # warm-up collective: absorb inter-core launch skew while
            # weights are still loading
            wone = work.tile([1, 1], f32, tag="wone")
            nc.vector.memset(wone[:], 1.0)
            nc.sync.dma_start(out=arw_in[:], in_=wone[:])
            nc.gpsimd.collective_compute(
                "AllReduce", ALU.add, ins=[arw_in[:]], outs=[arw_out[:]],
                replica_groups=grp_all)

            qkv_chunks(0, [0, 1, 2, 3, 4])
            vdirect(0, 0, F)
            halting(0)
            for si, s in enumerate(SCALES):
                advance(0, si, s, F - 1)   # fires AR groups from inside s=6

            qkv_chunks(1, [0, 1])          # waits on AR group 0 write-back
            vdirect(1, 0, 8)
            for si, s in enumerate(SCALES):
                advance(1, si, s, 8 - s)
            qkv_chunks(1, [2, 3])          # waits on AR group 1
            vdirect(1, 8, 16)
            for si, s in enumerate(SCALES):
                advance(1, si, s, 16 - s)
            qkv_chunks(1, [4])             # waits on AR group 2
            vdirect(1, 16, F)
            halting(1)
            # coefficient row (needs wts[1] from halting(1))
            cc_t = work.tile([18, 2], f32, tag="cc_t")
            tmpc = work.tile([18, 1], f32, tag="tmpc")
            nc.vector.tensor_copy(cc_t[:, 0:1], wts[1][:])
            nc.vector.tensor_scalar(out=tmpc[:], in0=wts[1][:], scalar1=-1.0,
                                    scalar2=1.0, op0=ALU.mult, op1=ALU.add)
            nc.vector.tensor_tensor(out=tmpc[:], in0=tmpc[:], in1=wts[0][:],
                                    op=ALU.mult)
            nc.vector.tensor_scalar_mul(out=cc_t[:, 1:2], in0=tmpc[:],
                                        scalar1=1.0 / NCORES)
            ccr = work.tile([1, 36], f32, tag="ccr")
            for k in range(2):
                pcc = ppT.tile([1, 18], f32, tag="pt", name="pcc")
                nc.tensor.transpose(pcc[:], cc_t[:, k:k + 1], ident[0:18, 0:18])
                nc.vector.tensor_copy(ccr[:, k * 18:(k + 1) * 18], pcc[:])
            pc = ppT.tile([128, 36], f32, tag="pt", name="pc")
            nc.tensor.matmul(pc[:], ones_row[:], ccr[:],
                             start=True, stop=True)
            coefb = work.tile([128, 36], f32, tag="coefb")
            nc.vector.tensor_copy(coefb[:], pc[:])

            for si, s in enumerate(SCALES):
                advance(1, si, s, F - 1)   # absum/ssum1 fire from checkpoints
            for hc in range(2):
                nc.vector.tensor_reduce(
                    out=ssum0[hc][:],
                    in_=xt[hc][:].rearrange("p (f q) -> p f q", q=128),
                    axis=AX.X, op=ALU.add)

            # ============ final combine ============
            vpart = [work.tile([128, 1], f32, tag=f"vpart{hc}",
                               name=f"vpart{hc}") for hc in range(2)]
            for hc in range(2):
                t2 = work.tile([128, F], f32, tag="t2")
                nc.vector.tensor_tensor(out=t2[:], in0=ssum1[hc][:],
                                        in1=coefb[:, 0:F], op=ALU.mult)
                t1 = work.tile([128, F], f32, tag="t1")
                nc.vector.tensor_tensor(out=t1[:], in0=ssum0[hc][:],
                                        in1=coefb[:, F:2 * F], op=ALU.mult)
                nc.vector.tensor_tensor(out=t2[:], in0=t2[:], in1=t1[:],
                                        op=ALU.add)
                nc.vector.tensor_reduce(out=vpart[hc][:], in_=t2[:],
                                        axis=AX.X, op=ALU.add)
                nc.sync.dma_start(out=ar2_in[hc], in_=vpart[hc][:])
            nc.gpsimd.collective_compute(
                "AllReduce", ALU.add,
                ins=[ar2_in[:]], outs=[ar2_out[:]],
                replica_groups=grp_all)
            vfull = [work.tile([128, 1], f32, tag=f"vfull{hc}",
                               name=f"vfull{hc}") for hc in range(2)]
            vfullb = [work.tile([128, 1], bf16, tag=f"vfullb{hc}",
                                name=f"vfullb{hc}") for hc in range(2)]
            ob = work.tile([1, NCLS], f32, tag="ob")
            for hc in range(2):
                nc.sync.dma_start(out=vfull[hc][:], in_=ar2_out[hc])
                nc.vector.tensor_copy(vfullb[hc][:], vfull[hc][:])
            off = 0
            for w_cc in _chunks(NCLS):
                pcls = ppq.tile([1, 512], f32, tag="pq", name="pcls")
                for hc in range(2):
                    nc.tensor.matmul(
                        pcls[:, :w_cc], vfullb[hc][:],
                        clsw[:, hc * NCLS + off:hc * NCLS + off + w_cc],
                        start=(hc == 0), stop=(hc == 1))
                nc.vector.tensor_tensor(out=ob[:, off:off + w_cc],
                                        in0=pcls[:, :w_cc],
                                        in1=clsb[:, off:off + w_cc], op=ALU.add)
                off += w_cc
            nc.sync.dma_start(out=out_d[:], in_=ob[:])

    nc.compile()
    return nc


_NC_CACHE = None


def _get_nc():
    global _NC_CACHE
    if _NC_CACHE is None:
        _NC_CACHE = build()
    return _NC_CACHE


def _prep_in_maps(inputs):
    emb = np.ascontiguousarray(np.asarray(inputs["multiscale_embed"], np.float32))
    halt_W = np.asarray(inputs["halt_W"], np.float32)
    halt_b = np.asarray(inputs["halt_b"], np.float32)
    cls_W = np.asarray(inputs["cls_W"], np.float32)
    cls_b = np.asarray(inputs["cls_b"], np.float32)
    Wq = np.asarray(inputs["mhsa_Wq"], np.float32)
    bq = np.asarray(inputs["mhsa_bq"], np.float32)
    Wk = np.asarray(inputs["mhsa_Wk"], np.float32)
    bk = np.asarray(inputs["mhsa_bk"], np.float32)
    Wv = np.asarray(inputs["mhsa_Wv"], np.float32)
    bv = np.asarray(inputs["mhsa_bv"], np.float32)
    Wo = np.asarray(inputs["mhsa_Wo"], np.float32)
    bo = np.asarray(inputs["mhsa_bo"], np.float32)

    import ml_dtypes
    bf = ml_dtypes.bfloat16
    xt = np.ascontiguousarray(
        emb.reshape(NTOK, H).T.reshape(2, 128, NTOK)).astype(bf)
    # AllReduce sums the per-token bias over 8 cores -> divide by NCORES
    boq = np.ascontiguousarray(
        (0.25 / NCORES * bo.sum(axis=0)).reshape(2, 128, 1))
    boqs = np.ascontiguousarray(
        (32.0 / NCORES * bo.sum(axis=0)).reshape(2, 128, 1))
    hwc = halt_W.reshape(2, 128)
    nhb = np.full((18, 1), float(halt_b[0]), np.float32)
    clsw = np.ascontiguousarray(cls_W.reshape(2, 128, NCLS)).astype(bf)
    clsb = cls_b.reshape(1, NCLS).astype(np.float32)
    ident = np.eye(128, dtype=np.float32)

    in_maps = []
    for h in range(NCORES):
        sl = slice(h * HD, (h + 1) * HD)
        wqkv = np.zeros((3, 2, 128, 65), bf)
        bqkv = np.zeros((3, 65, 1), np.float32)
        wo_l = np.zeros((3, 32, 256), bf)
        wv_l = np.zeros((2, 128, 96), bf)
        vb_l = np.zeros((128, 96), np.float32)
        for si in range(3):
            blk = np.concatenate([Wq[si][:, sl], Wk[si][:, sl]], axis=1)
            wqkv[si, :, :, :64] = blk.reshape(2, 128, 64).astype(bf)
            if si == 0:
                wqkv[si, :, :, 64] = hwc.astype(bf)
            bqkv[si, :64] = np.concatenate([bq[si][sl], bk[si][sl]])[:, None]
            wo_l[si] = Wo[si][sl, :].astype(bf)
            wv_l[:, :, si * 32:(si + 1) * 32] = \
                Wv[si][:, sl].reshape(2, 128, 32).astype(bf)
            vb_l[:, si * 32:(si + 1) * 32] = bv[si][sl][None, :]
        in_maps.append({
            "xt": xt, "wqkv": wqkv, "bqkv": bqkv, "wv": wv_l, "vb": vb_l,
            "wo": wo_l, "boq": boq, "boqs": boqs, "nhb": nhb,
            "clsw": clsw, "clsb": clsb, "ident": ident,
        })
    return in_maps


def run(inputs, trace=False):
    _install_ntff_hook()
    from concourse.bass_utils import run_bass_kernel_spmd

    nc = _get_nc()
    in_maps = _prep_in_maps(inputs)
    res = run_bass_kernel_spmd(nc, in_maps, list(range(NCORES)), trace=trace)
    out = np.asarray(res.results[0]["out"], np.float32)
    return out, res


def kernel(**inputs):
    out, _ = run(inputs, trace=False)
    return out


# revision 42
# speedup vs baseline: 1.0224x; 1.0224x over previous
"""Trainium2 Bass kernel for nn_CRF_SelfAttention_65627100283470 (v2).

Math (validated vs the reference at 1e-6 rel err):
  - The CRF/marginal branch is dead code: softmax over the class dim sums
    to 1, so sum(cluster_features, 0) == sum of context rows.  The output
    is (sum_{f,p} context2) @ cls_W + cls_b.
  - context2 = w2*T2 + w1*(1-w2)*T1 -> only per-frame sums of temporal are
    needed at the end; iteration 2 therefore skips the full Wo projection
    and uses per-frame sums of the attention outputs instead.
  - QKV projections are shared across overlapping windows; exp(scores)
    blocks are shared across windows (computed per key-frame strip).

v2 kernel structure (per core = one head):
  - A@V' is computed per (key-frame, query-frame) PAIR: the exp-score
    block [128k,128q] is the matmul's stationary operand and V'[kf]
    [128,33] streams through (33 cols), so each block is streamed ONCE
    and the output lands token-major (no transposes, no per-window
    re-streaming).  Per-window softmax sums are sliding-window sums over
    kf, computed as 2-3 full-width strided adds on the Pool engine.
  - V' is projected directly token-major (lhsT = x-block), batched over
    the 3 scales in one matmul per (frame, k-chunk).
  - Halting probabilities live in [18,128] layout (one DMA scatter), so
    the per-frame mean needs no per-frame transposes.
  - The inter-iteration AllReduce is chunked (3 chunks) and overlapped
    with the Wo projection feeding it.

Sharding: 8 heads -> 8 cores. One chunked AllReduce of the partial
temporal between the two iterations + one tiny final AllReduce.
"""
import sys
import types

import numpy as np

F, P, H, HEADS, C, NCLS = 18, 128, 256, 8, 32, 625
SCALES = (2, 4, 6)
HD = H // HEADS
NTOK = F * P  # 2304
NCORES = 8


def _enable_ldw_opt():
    """Walrus's LDWEIGHTS dedup is disabled by default in bass_utils;
    enable it (verified numerically by the rel-err gate in test.py)."""
    import concourse.bass_utils as bu

    if getattr(bu, "_ldw_opt_patched", False):
        return
    orig = bu.bir_verify_and_optimise

    def patched(*args, **kwargs):
        real_run = bu.run_command

        def run_hook(argv, **kw):
            argv = ["--enable-ldw-opt=true" if a == "--enable-ldw-opt=false"
                    else a for a in argv]
            return real_run(argv, **kw)

        bu.run_command = run_hook
        try:
            return orig(*args, **kwargs)
        finally:
            bu.run_command = real_run

    bu.bir_verify_and_optimise = patched
    bu._ldw_opt_patched = True


def _install_ntff_hook():
    """Recreate the missing antenv.axon_hooks so trace=True works."""
    if "antenv.axon_hooks" in sys.modules:
        return
    try:
        import antenv

        mod = types.ModuleType("antenv.axon_hooks")
        mod._hook = None
        mod.set_axon_ntff_profile_hook = lambda h: setattr(mod, "_hook", h)
        mod.get_axon_ntff_profile_hook = lambda: mod._hook
        sys.modules["antenv.axon_hooks"] = mod
        antenv.axon_hooks = mod
        from trn_agent_boot.trn_boot import _ntff_profile_via_ctypes

        mod.set_axon_ntff_profile_hook(
            _ntff_profile_via_ctypes("/opt/axon/libaxon_pjrt.so")
        )
    except Exception:
        pass


def _chunks(n, lim=512):
    out = [lim] * (n // lim)
    if n % lim:
        out.append(n % lim)
    return out


def _counts(s):
    nw = F - s + 1
    c = np.zeros(F, np.float32)
    for w in range(nw):
        c[w:w + s] += 1.0
    return c


COL_CC = _chunks(NTOK)          # [512,512,512,512,256]
AR_GROUPS = [(0, 1), (2, 3), (4,)]  # chunk-index groups per AllReduce
GROUP_FRAMES = [(0, 8), (8, 16), (16, 18)]  # frame range per AR group


def build():
    import concourse.bacc as bacc
    import concourse.mybir as mybir
    from concourse.tile import TileContext

    dt = mybir.dt
    f32 = dt.float32
    bf16 = dt.bfloat16
    f8 = dt.float8e4
    AF = mybir.ActivationFunctionType
    ALU = mybir.AluOpType
    AX = mybir.AxisListType

    nc = bacc.Bacc("TRN2", target_bir_lowering=False, debug=False,
                   num_devices=NCORES)

    # ---- I/O ----
    xt_in = nc.dram_tensor("xt", [2, 128, NTOK], bf16, kind="ExternalInput")
    wqkv_in = nc.dram_tensor("wqkv", [3, 2, 128, 65], bf16, kind="ExternalInput")
    bqkv_in = nc.dram_tensor("bqkv", [3, 65, 1], f32, kind="ExternalInput")
    wv_in = nc.dram_tensor("wv", [2, 128, 96], bf16, kind="ExternalInput")
    vb_in = nc.dram_tensor("vb", [128, 96], f32, kind="ExternalInput")
    wo_in = nc.dram_tensor("wo", [3, 32, 256], bf16, kind="ExternalInput")
    boq_in = nc.dram_tensor("boq", [2, 128, 1], f32, kind="ExternalInput")
    boqs_in = nc.dram_tensor("boqs", [2, 128, 1], f32, kind="ExternalInput")
    nhb_in = nc.dram_tensor("nhb", [18, 1], f32, kind="ExternalInput")
    clsw_in = nc.dram_tensor("clsw", [2, 128, NCLS], bf16, kind="ExternalInput")
    clsb_in = nc.dram_tensor("clsb", [1, NCLS], f32, kind="ExternalInput")
    id_in = nc.dram_tensor("ident", [128, 128], f32, kind="ExternalInput")
    out_d = nc.dram_tensor("out", [1, NCLS], f32, kind="ExternalOutput")

    ar_in_c = []
    ar_out_c = []
    for gi, grp in enumerate(AR_GROUPS):
        w = sum(COL_CC[ci] for ci in grp)
        ar_in_c.append(nc.dram_tensor(f"ar_in{gi}", [2, 128, w], bf16))
        ar_out_c.append(nc.dram_tensor(f"ar_out{gi}", [2, 128, w], bf16,
                                       addr_space="Shared"))
    ar2_in = nc.dram_tensor("ar2_in", [2, 128, 1], f32)
    ar2_out = nc.dram_tensor("ar2_out", [2, 128, 1], f32, addr_space="Shared")
    arw_in = nc.dram_tensor("arw_in", [1, 1], f32)
    arw_out = nc.dram_tensor("arw_out", [1, 1], f32, addr_space="Shared")
    hrow_d = nc.dram_tensor("hrow", [2, 1, NTOK], bf16)
    absd = nc.dram_tensor("absd", [3, 2, 1, 288], bf16)

    inv_sqrt_hd = 1.0 / np.sqrt(np.float32(HD))
    cnts = {s: _counts(s) for s in SCALES}
    grp_all = [list(range(NCORES))]

    with TileContext(nc) as tc:
        with (
            tc.tile_pool(name="pin", bufs=1) as pin,
            tc.tile_pool(name="work", bufs=3) as work,
            tc.tile_pool(name="tree", bufs=3) as tree,
            tc.tile_pool(name="estr2", bufs=4) as estr2,
            tc.tile_pool(name="estr4", bufs=8) as estr4,
            tc.tile_pool(name="estr6", bufs=12) as estr6,
            tc.tile_pool(name="ppq", bufs=2, space="PSUM") as ppq,
            tc.tile_pool(name="pstr", bufs=3, space="PSUM") as pstr,
            tc.tile_pool(name="ppT", bufs=3, space="PSUM") as ppT,
        ):
            estr = {2: estr2, 4: estr4, 6: estr6}

            # ---- persistent tiles + weight loads ----
            xt = [pin.tile([128, NTOK], bf16, tag=f"xt{c}", name=f"xt{c}")
                  for c in range(2)]
            wqkv = pin.tile([128, 3 * 2 * 65], bf16, tag="wqkv")
            bqkv = pin.tile([65, 3], f32, tag="bqkv")
            wv = pin.tile([128, 2 * 96], bf16, tag="wv")
            vb = pin.tile([128, 96], f32, tag="vb")
            wo = pin.tile([32, 3 * 256], bf16, tag="wo")
            boq = pin.tile([128, 2], f32, tag="boq")
            boqs = pin.tile([128, 2], f32, tag="boqs")
            nhb = pin.tile([18, 1], f32, tag="nhb")
            clsw = pin.tile([128, 2 * NCLS], bf16, tag="clsw")
            clsb = pin.tile([1, NCLS], f32, tag="clsb")
            ident = pin.tile([128, 128], f32, tag="ident")
            identb = pin.tile([128, 128], bf16, tag="identb")
            ones_row = pin.tile([1, 128], f32, tag="ones_row")
            ones_col = pin.tile([128, 1], f32, tag="ones_col")
            ones_colb = pin.tile([128, 1], bf16, tag="ones_colb")

            for c in range(2):
                off = 0
                for w_cc in COL_CC:
                    nc.sync.dma_start(out=xt[c][:, off:off + w_cc],
                                      in_=xt_in[c, :, off:off + w_cc])
                    off += w_cc
            for si in range(3):
                for c in range(2):
                    nc.sync.dma_start(
                        out=wqkv[:, (si * 2 + c) * 65:(si * 2 + c + 1) * 65],
                        in_=wqkv_in[si, c])
                nc.sync.dma_start(out=bqkv[:, si:si + 1], in_=bqkv_in[si])
                nc.gpsimd.dma_start(out=wo[:, si * 256:(si + 1) * 256],
                                    in_=wo_in[si])
            for c in range(2):
                nc.gpsimd.dma_start(out=wv[:, c * 96:(c + 1) * 96], in_=wv_in[c])
                nc.gpsimd.dma_start(out=boq[:, c:c + 1], in_=boq_in[c])
                nc.gpsimd.dma_start(out=boqs[:, c:c + 1], in_=boqs_in[c])
                nc.gpsimd.dma_start(out=clsw[:, c * NCLS:(c + 1) * NCLS],
                                    in_=clsw_in[c])
            nc.sync.dma_start(out=vb[:], in_=vb_in[:])
            nc.sync.dma_start(out=nhb[:], in_=nhb_in[:])
            nc.gpsimd.dma_start(out=clsb[:], in_=clsb_in[:])
            nc.gpsimd.dma_start(out=ident[:], in_=id_in[:])
            nc.vector.memset(ones_row[:], 1.0)
            nc.vector.memset(ones_col[:], 1.0)
            nc.vector.memset(ones_colb[:], 1.0)
            nc.vector.tensor_copy(identb[:], ident[:])

            qkvT = {s: pin.tile([65, NTOK], bf16, tag=f"qkvT{s}",
                                name=f"qkvT{s}") for s in SCALES}
            kT = {s: pin.tile([32, NTOK], bf16, tag=f"kT{s}",
                              name=f"kT{s}") for s in SCALES}
            # V' per frame: [V_s2|1|V_s4|1|V_s6|1] -> 99 cols, ones persist
            vp = pin.tile([128, F * 99], bf16, tag="vp")
            nc.vector.memset(vp[:], 1.0)
            abar = {s: pin.tile([128, F * 32], f32, tag=f"abar{s}",
                                name=f"abar{s}") for s in SCALES}
            abarb = {s: pin.tile([128, F * 32], bf16, tag=f"abarb{s}",
                                 name=f"abarb{s}") for s in SCALES}
            abarT = {s: pin.tile([32, NTOK], bf16, tag=f"abarT{s}",
                                 name=f"abarT{s}") for s in SCALES}
            abs32 = {s: pin.tile([32, F], bf16, tag=f"abs32{s}",
                                 name=f"abs32{s}") for s in SCALES}
            ssum0 = [pin.tile([128, F], f32, tag=f"ssum0{c}", name=f"ssum0{c}")
                     for c in range(2)]
            ssum1 = [pin.tile([128, F], f32, tag=f"ssum1{c}", name=f"ssum1{c}")
                     for c in range(2)]

            # halting state, column layout [18,1]
            ptn = pin.tile([18, 1], f32, tag="ptn")
            Rt = pin.tile([18, 1], f32, tag="Rt")
            wts = [pin.tile([18, 1], f32, tag=f"w{it}", name=f"w{it}")
                   for it in range(2)]
            nc.vector.memset(ptn[:], 0.0)
            nc.vector.memset(Rt[:], 0.0)

            # ---------------- schedule closures ----------------
            prog = {}      # (it, s) -> next kf to process
            stripmap = {}  # (it, s) -> {kf: (a_k, est_tile)}

            def qkv_chunks(it, cis):
                for si, s in enumerate(SCALES):
                    for ci in cis:
                        off = sum(COL_CC[cj] for cj in range(ci))
                        w_cc = COL_CC[ci]
                        pq = ppq.tile([65, 512], f32, tag="pq", name="pq")
                        for kc in range(2):
                            nc.tensor.matmul(
                                pq[:, :w_cc],
                                wqkv[:, (si * 2 + kc) * 65:(si * 2 + kc + 1) * 65],
                                xt[kc][:, off:off + w_cc],
                                start=(kc == 0), stop=(kc == 1))
                        nc.scalar.activation(qkvT[s][:, off:off + w_cc],
                                             pq[:, :w_cc], AF.Identity,
                                             bias=bqkv[:, si:si + 1], scale=1.0)
                        nc.sync.dma_start(out=kT[s][:, off:off + w_cc],
                                          in_=qkvT[s][32:64, off:off + w_cc])

            def vdirect(it, f0, f1):
                for f in range(f0, f1):
                    pv = ppT.tile([128, 363], f32, tag="pt", name="pv")
                    for kc in range(2):
                        nc.tensor.matmul(
                            pv[:, :96],
                            xt[kc][:, f * 128:(f + 1) * 128],
                            wv[:, kc * 96:(kc + 1) * 96],
                            start=(kc == 0), stop=(kc == 1))
                    vslc = vp[:, f * 99:(f + 1) * 99].rearrange(
                        "p (s c) -> p s c", c=33)
                    nc.vector.tensor_tensor(
                        out=vslc[:, :, 0:32],
                        in0=pv[:, :96].rearrange("p (s c) -> p s c", c=32),
                        in1=vb[:].rearrange("p (s c) -> p s c", c=32),
                        op=ALU.add)

            def halting(it):
                hl = work.tile([18, 128], bf16, tag="hl")
                nc.sync.dma_start(out=hrow_d[it], in_=qkvT[2][64:65, :])
                nc.sync.dma_start(
                    out=hl[:],
                    in_=hrow_d[it].rearrange("o (f q) -> (o f) q", q=128))
                sg = work.tile([18, 128], f32, tag="sg")
                nc.scalar.activation(sg[:], hl[:], AF.Sigmoid,
                                     bias=nhb[:], scale=1.0)
                p_t = work.tile([18, 1], f32, tag="p_t")
                nc.vector.tensor_reduce(out=p_t[:], in_=sg[:],
                                        axis=AX.X, op=ALU.add)
                nc.vector.tensor_scalar_mul(out=p_t[:], in0=p_t[:],
                                            scalar1=1.0 / 128.0)
                run_in = work.tile([18, 1], f32, tag="run_in")
                tmp = work.tile([18, 1], f32, tag="tmp")
                tmp2 = work.tile([18, 1], f32, tag="tmp2")
                nh = work.tile([18, 1], f32, tag="nh")
                run = work.tile([18, 1], f32, tag="run")
                nc.vector.tensor_scalar(out=run_in[:], in0=ptn[:], scalar1=1.0,
                                        scalar2=None, op0=ALU.is_lt)
                nc.vector.tensor_tensor(out=tmp[:], in0=p_t[:], in1=run_in[:],
                                        op=ALU.mult)
                nc.vector.tensor_tensor(out=tmp2[:], in0=ptn[:], in1=tmp[:],
                                        op=ALU.add)
                nc.vector.tensor_scalar(out=tmp2[:], in0=tmp2[:], scalar1=0.99,
                                        scalar2=None, op0=ALU.is_gt)
                nc.vector.tensor_tensor(out=nh[:], in0=tmp2[:], in1=run_in[:],
                                        op=ALU.mult)
                nc.vector.tensor_tensor(out=run[:], in0=run_in[:], in1=nh[:],
                                        op=ALU.subtract)
                nc.vector.tensor_tensor(out=tmp[:], in0=p_t[:], in1=run[:],
                                        op=ALU.mult)
                nc.vector.tensor_tensor(out=ptn[:], in0=ptn[:], in1=tmp[:],
                                        op=ALU.add)
                nc.vector.tensor_scalar(out=tmp2[:], in0=ptn[:], scalar1=-1.0,
                                        scalar2=1.0, op0=ALU.mult, op1=ALU.add)
                nc.vector.tensor_tensor(out=tmp2[:], in0=nh[:], in1=tmp2[:],
                                        op=ALU.mult)
                nc.vector.tensor_tensor(out=Rt[:], in0=Rt[:], in1=tmp2[:],
                                        op=ALU.add)
                nc.vector.tensor_tensor(out=tmp2[:], in0=nh[:], in1=Rt[:],
                                        op=ALU.mult)
                nc.vector.tensor_tensor(out=ptn[:], in0=ptn[:], in1=tmp2[:],
                                        op=ALU.add)
                nc.vector.tensor_tensor(out=wts[it][:], in0=tmp[:],
                                        in1=tmp2[:], op=ALU.add)

            def fire_group(gi):
                grp = AR_GROUPS[gi]
                f0, f1 = GROUP_FRAMES[gi]
                for sx in SCALES:
                    nc.scalar.copy(abarb[sx][:, f0 * 32:f1 * 32],
                                   abar[sx][:, f0 * 32:f1 * 32])
                    for f in range(f0, f1):
                        pat = ppT.tile([32, 128], bf16, tag="pt", name="pat")
                        nc.tensor.transpose(pat[:],
                                            abarb[sx][:, f * 32:(f + 1) * 32],
                                            identb[:])
                        nc.vector.tensor_copy(
                            abarT[sx][:, f * 128:(f + 1) * 128], pat[:])
                goff = sum(COL_CC[ci] for ci in range(grp[0]))
                for ci in grp:
                    off = sum(COL_CC[cj] for cj in range(ci))
                    w_cc = COL_CC[ci]
                    for hc in range(2):
                        pw = ppq.tile([128, 512], f32, tag="pq", name="pw")
                        for sj, sx in enumerate(SCALES):
                            nc.tensor.matmul(
                                pw[:, :w_cc],
                                wo[:, sj * 256 + hc * 128:sj * 256 + (hc + 1) * 128],
                                abarT[sx][:, off:off + w_cc],
                                start=(sj == 0), stop=(sj == 2))
                        nc.vector.tensor_scalar(
                            out=xt[hc][:, off:off + w_cc],
                            in0=pw[:, :w_cc],
                            scalar1=0.25, scalar2=boq[:, hc:hc + 1],
                            op0=ALU.mult, op1=ALU.add)
                        nc.sync.dma_start(
                            out=ar_in_c[gi][hc, :, off - goff:off - goff + w_cc],
                            in_=xt[hc][:, off:off + w_cc])
                nc.gpsimd.collective_compute(
                    "AllReduce", ALU.add,
                    ins=[ar_in_c[gi][:]], outs=[ar_out_c[gi][:]],
                    replica_groups=grp_all)
                gw = sum(COL_CC[ci] for ci in grp)
                for hc in range(2):
                    nc.sync.dma_start(out=xt[hc][:, goff:goff + gw],
                                      in_=ar_out_c[gi][hc])

            def advance(it, si, s, kf_limit):
                strips = stripmap.setdefault((it, s), {})
                for kf in range(prog.get((it, s), 0), kf_limit + 1):
                    a_k = max(0, kf - s + 1)
                    b_k = min(F - 1, kf + s - 1)
                    ncols = (b_k - a_k + 1) * 128
                    est_t = estr[s].tile([128, (2 * s - 1) * 128], f8,
                                         tag="est")
                    off = 0
                    for w_cc in _chunks(ncols):
                        ps = pstr.tile([128, 512], f32, tag="ps", name="ps")
                        nc.tensor.matmul(
                            ps[:, :w_cc],
                            kT[s][:, kf * 128:(kf + 1) * 128],
                            qkvT[s][0:32, a_k * 128 + off:a_k * 128 + off + w_cc],
                            start=True, stop=True)
                        nc.scalar.activation(est_t[:, off:off + w_cc],
                                             ps[:, :w_cc], AF.Exp,
                                             scale=inv_sqrt_hd)
                        off += w_cc
                    strips[kf] = (a_k, est_t)

                    if s - 1 <= kf < F - 1:
                        qlist = [kf - s + 1]
                    elif kf == F - 1:
                        qlist = list(range(F - s, F))
                    else:
                        qlist = []

                    for qi, qf in enumerate(qlist):
                        te = nc.vector
                        a = max(0, qf - s + 1)
                        b = min(F - 1, qf + s - 1)
                        W = b - a + 1
                        nwq = W - s + 1
                        pT = ppT.tile([128, 363], f32, tag="pt", name="pT")
                        for j in range(W):
                            kfj = a + j
                            ak_j, est_j = strips[kfj]
                            qo = (qf - ak_j) * 128
                            nc.tensor.matmul(
                                pT[:, j * 33:(j + 1) * 33],
                                est_j[:, qo:qo + 128],
                                vp[:, kfj * 99 + si * 33:kfj * 99 + si * 33 + 33],
                                start=True, stop=True)
                        # sliding-window sums over kf
                        Tt = tree.tile([128, 363], f32, tag="Tt")
                        nc.scalar.copy(Tt[:, :W * 33], pT[:, :W * 33])
                        U = tree.tile([128, 330], f32, tag="U")
                        te.tensor_tensor(
                            out=U[:, :(W - 1) * 33],
                            in0=Tt[:, 0:(W - 1) * 33],
                            in1=Tt[:, 33:W * 33], op=ALU.add)
                        if s == 2:
                            S_ap = U
                        else:
                            Q = tree.tile([128, 264], f32, tag="Q")
                            te.tensor_tensor(
                                out=Q[:, :(W - 3) * 33],
                                in0=U[:, 0:(W - 3) * 33],
                                in1=U[:, 66:(W - 1) * 33], op=ALU.add)
                            if s == 4:
                                S_ap = Q
                            else:
                                S6 = tree.tile([128, 198], f32, tag="S")
                                te.tensor_tensor(
                                    out=S6[:, :(W - 5) * 33],
                                    in0=Q[:, 0:(W - 5) * 33],
                                    in1=U[:, 132:(W - 1) * 33], op=ALU.add)
                                S_ap = S6
                        # normalize per window, weight by 1/count, sum
                        Sv = S_ap[:, :nwq * 33].rearrange(
                            "p (w c) -> p w c", c=33)
                        rcp = tree.tile([128, 6], f32, tag="rcp")
                        nc.vector.reciprocal(rcp[:, :nwq], Sv[:, :, 32])
                        cinv = float(1.0 / cnts[s][qf])
                        ab_out = abar[s][:, qf * 32:(qf + 1) * 32]
                        if nwq == 1:
                            nc.vector.tensor_scalar(
                                out=ab_out, in0=S_ap[:, 0:32],
                                scalar1=rcp[:, 0:1], scalar2=cinv,
                                op0=ALU.mult, op1=ALU.mult)
                        else:
                            sc = tree.tile([128, 192], f32, tag="sc")
                            scv = sc[:, :nwq * 32].rearrange(
                                "p (w c) -> p w c", c=32)
                            nc.vector.scalar_tensor_tensor(
                                out=scv, in0=Sv[:, :, 0:32], scalar=cinv,
                                in1=rcp[:, :nwq].broadcast_to((128, nwq, 32)),
                                op0=ALU.mult, op1=ALU.mult)
                            m = nwq
                            while m > 2:
                                h = m // 2
                                te.tensor_tensor(
                                    out=sc[:, (m - 2 * h) * 32:(m - h) * 32],
                                    in0=sc[:, (m - 2 * h) * 32:(m - h) * 32],
                                    in1=sc[:, (m - h) * 32:m * 32],
                                    op=ALU.add)
                                m -= h
                            te.tensor_tensor(
                                out=ab_out, in0=sc[:, 0:32],
                                in1=sc[:, 32:64], op=ALU.add)

                        if it == 0 and s == 6:
                            if qf == 7:
                                fire_group(0)
                            elif qf == 15:
                                fire_group(1)
                            elif qf == 17:
                                fire_group(2)
                        elif it == 1:
                            if qf == 8:
                                absum_half(si, s, 0)
                            elif qf == 17:
                                absum_half(si, s, 1)
                                ssum1_accum(si, s)
                prog[(it, s)] = kf_limit + 1

            def absum_half(si, s, half):
                abh = work.tile([128, 288], bf16, tag="abh")
                nc.vector.tensor_copy(abh[:],
                                      abar[s][:, half * 288:(half + 1) * 288])
                pa = ppq.tile([1, 512], f32, tag="pq", name="pa")
                nc.tensor.matmul(
                    pa[:, :288], ones_colb[:], abh[:],
                    start=True, stop=True)
                asb = work.tile([1, 288], bf16, tag="asb")
                nc.vector.tensor_copy(asb[:], pa[:, :288])
                nc.sync.dma_start(out=absd[si, half], in_=asb[0:1, :])
                nc.sync.dma_start(
                    out=abs32[s][:, half * 9:(half + 1) * 9],
                    in_=absd[si, half].rearrange(
                        "o (f c) -> (o c) f", c=32))

            def ssum1_accum(si, s):
                for hc in range(2):
                    pw1 = ppT.tile([128, 363], f32, tag="pt", name="pw1")
                    nc.tensor.matmul(
                        pw1[:, :F],
                        wo[:, si * 256 + hc * 128:si * 256 + (hc + 1) * 128],
                        abs32[s][:], start=True, stop=True)
                    if si == 0:
                        nc.vector.tensor_scalar(
                            out=ssum1[hc][:], in0=pw1[:, :F],
                            scalar1=0.25, scalar2=boqs[:, hc:hc + 1],
                            op0=ALU.mult, op1=ALU.add)
                    else:
                        nc.vector.scalar_tensor_tensor(
                            out=ssum1[hc][:], in0=pw1[:, :F], scalar=0.25,
                            in1=ssum1[hc][:], op0=ALU.mult, op1=ALU.add)

            # ---------------- schedule ----------------
            # warm-up collective: absorb inter-core launch skew while
            # weights are still loading
            wone = work.tile([1, 1], f32, tag="wone")
            nc.vector.memset(wone[:], 1.0)
            nc.sync.dma_start(out=arw_in[:], in_=wone[:])
            nc.gpsimd.collective_compute(
                "AllReduce", ALU.add, ins=[arw_in[:]], outs=[arw_out[:]],
                replica_groups=grp_all)

            qkv_chunks(0, [0, 1, 2, 3, 4])
            vdirect(0, 0, F)
            halting(0)
            for si, s in enumerate(SCALES):
                advance(0, si, s, F - 1)   # fires AR groups from inside s=6

            qkv_chunks(1, [0, 1])          # waits on AR group 0 write-back
            vdirect(1, 0, 8)
            for si, s in enumerate(SCALES):
                advance(1, si, s, 8 - s)
            qkv_chunks(1, [2, 3])          # waits on AR group 1
            vdirect(1, 8, 16)
            for si, s in enumerate(SCALES):
                advance(1, si, s, 16 - s)
            qkv_chunks(1, [4])             # waits on AR group 2
            vdirect(1, 16, F)
            halting(1)
            # coefficient row (needs wts[1] from halting(1))
            cc_t = work.tile([18, 2], f32, tag="cc_t")
            tmpc = work.tile([18, 1], f32, tag="tmpc")
            nc.vector.tensor_copy(cc_t[:, 0:1], wts[1][:])
            nc.vector.tensor_scalar(out=tmpc[:], in0=wts[1][:], scalar1=-1.0,
                                    scalar2=1.0, op0=ALU.mult, op1=ALU.add)
            nc.vector.tensor_tensor(out=tmpc[:], in0=tmpc[:], in1=wts[0][:],
                                    op=ALU.mult)
            nc.vector.tensor_scalar_mul(out=cc_t[:, 1:2], in0=tmpc[:],
                                        scalar1=1.0 / NCORES)
            ccr = work.tile([1, 36], f32, tag="ccr")
            for k in range(2):
                pcc = ppT.tile([1, 18], f32, tag="pt", name="pcc")
                nc.tensor.transpose(pcc[:], cc_t[:, k:k + 1], ident[0:18, 0:18])
                nc.vector.tensor_copy(ccr[:, k * 18:(k + 1) * 18], pcc[:])
            pc = ppT.tile([128, 36], f32, tag="pt", name="pc")
            nc.tensor.matmul(pc[:], ones_row[:], ccr[:],
                             start=True, stop=True)
            coefb = work.tile([128, 36], f32, tag="coefb")
            nc.vector.tensor_copy(coefb[:], pc[:])

            for si, s in enumerate(SCALES):
                advance(1, si, s, F - 1)   # absum/ssum1 fire from checkpoints
            for hc in range(2):
                nc.vector.tensor_reduce(
                    out=ssum0[hc][:],
                    in_=xt[hc][:].rearrange("p (f q) -> p f q", q=128),
                    axis=AX.X, op=ALU.add)

            # ============ final combine ============
            vpart = [work.tile([128, 1], f32, tag=f"vpart{hc}",
                               name=f"vpart{hc}") for hc in range(2)]
            for hc in range(2):
                t2 = work.tile([128, F], f32, tag="t2")
                nc.vector.tensor_tensor(out=t2[:], in0=ssum1[hc][:],
                                        in1=coefb[:, 0:F], op=ALU.mult)
                t1 = work.tile([128, F], f32, tag="t1")
                nc.vector.tensor_tensor(out=t1[:], in0=ssum0[hc][:],
                                        in1=coefb[:, F:2 * F], op=ALU.mult)
                nc.vector.tensor_tensor(out=t2[:], in0=t2[:], in1=t1[:],
                                        op=ALU.add)
                nc.vector.tensor_reduce(out=vpart[hc][:], in_=t2[:],
                                        axis=AX.X, op=ALU.add)
                nc.sync.dma_start(out=ar2_in[hc], in_=vpart[hc][:])
            nc.gpsimd.collective_compute(
                "AllReduce", ALU.add,
                ins=[ar2_in[:]], outs=[ar2_out[:]],
                replica_groups=grp_all)
            vfull = [work.tile([128, 1], f32, tag=f"vfull{hc}",
                               name=f"vfull{hc}") for hc in range(2)]
            vfullb = [work.tile([128, 1], bf16, tag=f"vfullb{hc}",
                                name=f"vfullb{hc}") for hc in range(2)]
            ob = work.tile([1, NCLS], f32, tag="ob")
            for hc in range(2):
                nc.sync.dma_start(out=vfull[hc][:], in_=ar2_out[hc])
                nc.vector.tensor_copy(vfullb[hc][:], vfull[hc][:])
            off = 0
            for w_cc in _chunks(NCLS):
                pcls = ppq.tile([1, 512], f32, tag="pq", name="pcls")
                for hc in range(2):
                    nc.tensor.matmul(
                        pcls[:, :w_cc], vfullb[hc][:],
                        clsw[:, hc * NCLS + off:hc * NCLS + off + w_cc],
                        start=(hc == 0), stop=(hc == 1))
                nc.vector.tensor_tensor(out=ob[:, off:off + w_cc],
                                        in0=pcls[:, :w_cc],
                                        in1=clsb[:, off:off + w_cc], op=ALU.add)
                off += w_cc
            nc.sync.dma_start(out=out_d[:], in_=ob[:])

    nc.compile()
    return nc


_NC_CACHE = None


def _get_nc():
    global _NC_CACHE
    if _NC_CACHE is None:
        _NC_CACHE = build()
    return _NC_CACHE


def _prep_in_maps(inputs):
    emb = np.ascontiguousarray(np.asarray(inputs["multiscale_embed"], np.float32))
    halt_W = np.asarray(inputs["halt_W"], np.float32)
    halt_b = np.asarray(inputs["halt_b"], np.float32)
    cls_W = np.asarray(inputs["cls_W"], np.float32)
    cls_b = np.asarray(inputs["cls_b"], np.float32)
    Wq = np.asarray(inputs["mhsa_Wq"], np.float32)
    bq = np.asarray(inputs["mhsa_bq"], np.float32)
    Wk = np.asarray(inputs["mhsa_Wk"], np.float32)
    bk = np.asarray(inputs["mhsa_bk"], np.float32)
    Wv = np.asarray(inputs["mhsa_Wv"], np.float32)
    bv = np.asarray(inputs["mhsa_bv"], np.float32)
    Wo = np.asarray(inputs["mhsa_Wo"], np.float32)
    bo = np.asarray(inputs["mhsa_bo"], np.float32)

    import ml_dtypes
    bf = ml_dtypes.bfloat16
    xt = np.ascontiguousarray(
        emb.reshape(NTOK, H).T.reshape(2, 128, NTOK)).astype(bf)
    # AllReduce sums the per-token bias over 8 cores -> divide by NCORES
    boq = np.ascontiguousarray(
        (0.25 / NCORES * bo.sum(axis=0)).reshape(2, 128, 1))
    boqs = np.ascontiguousarray(
        (32.0 / NCORES * bo.sum(axis=0)).reshape(2, 128, 1))
    hwc = halt_W.reshape(2, 128)
    nhb = np.full((18, 1), float(halt_b[0]), np.float32)
    clsw = np.ascontiguousarray(cls_W.reshape(2, 128, NCLS)).astype(bf)
    clsb = cls_b.reshape(1, NCLS).astype(np.float32)
    ident = np.eye(128, dtype=np.float32)

    in_maps = []
    for h in range(NCORES):
        sl = slice(h * HD, (h + 1) * HD)
        wqkv = np.zeros((3, 2, 128, 65), bf)
        bqkv = np.zeros((3, 65, 1), np.float32)
        wo_l = np.zeros((3, 32, 256), bf)
        wv_l = np.zeros((2, 128, 96), bf)
        vb_l = np.zeros((128, 96), np.float32)
        for si in range(3):
            blk = np.concatenate([Wq[si][:, sl], Wk[si][:, sl]], axis=1)
            wqkv[si, :, :, :64] = blk.reshape(2, 128, 64).astype(bf)
            if si == 0:
                wqkv[si, :, :, 64] = hwc.astype(bf)
            bqkv[si, :64] = np.concatenate([bq[si][sl], bk[si][sl]])[:, None]
            wo_l[si] = Wo[si][sl, :].astype(bf)
            wv_l[:, :, si * 32:(si + 1) * 32] = \
                Wv[si][:, sl].reshape(2, 128, 32).astype(bf)
            vb_l[:, si * 32:(si + 1) * 32] = bv[si][sl][None, :]
        in_maps.append({
            "xt": xt, "wqkv": wqkv, "bqkv": bqkv, "wv": wv_l, "vb": vb_l,
            "wo": wo_l, "boq": boq, "boqs": boqs, "nhb": nhb,
            "clsw": clsw, "clsb": clsb, "ident": ident,
        })
    return in_maps


def run(inputs, trace=False):
    _install_ntff_hook()
    from concourse.bass_utils import run_bass_kernel_spmd

    nc = _get_nc()
    in_maps = _prep_in_maps(inputs)
    res = run_bass_kernel_spmd(nc, in_maps, list(range(NCORES)), trace=trace)
    out = np.asarray(res.results[0]["out"], np.float32)
    return out, res


def kernel(**inputs):
    out, _ = run(inputs, trace=False)
    return out


# revision 43
# speedup vs baseline: 1.0664x; 1.0431x over previous
"""Trainium2 Bass kernel for nn_CRF_SelfAttention_65627100283470 (v2).

Math (validated vs the reference at 1e-6 rel err):
  - The CRF/marginal branch is dead code: softmax over the class dim sums
    to 1, so sum(cluster_features, 0) == sum of context rows.  The output
    is (sum_{f,p} context2) @ cls_W + cls_b.
  - context2 = w2*T2 + w1*(1-w2)*T1 -> only per-frame sums of temporal are
    needed at the end; iteration 2 therefore skips the full Wo projection
    and uses per-frame sums of the attention outputs instead.
  - QKV projections are shared across overlapping windows; exp(scores)
    blocks are shared across windows (computed per key-frame strip).

v2 kernel structure (per core = one head):
  - A@V' is computed per (key-frame, query-frame) PAIR: the exp-score
    block [128k,128q] is the matmul's stationary operand and V'[kf]
    [128,33] streams through (33 cols), so each block is streamed ONCE
    and the output lands token-major (no transposes, no per-window
    re-streaming).  Per-window softmax sums are sliding-window sums over
    kf, computed as 2-3 full-width strided adds on the Pool engine.
  - V' is projected directly token-major (lhsT = x-block), batched over
    the 3 scales in one matmul per (frame, k-chunk).
  - Halting probabilities live in [18,128] layout (one DMA scatter), so
    the per-frame mean needs no per-frame transposes.
  - The inter-iteration AllReduce is chunked (3 chunks) and overlapped
    with the Wo projection feeding it.

Sharding: 8 heads -> 8 cores. One chunked AllReduce of the partial
temporal between the two iterations + one tiny final AllReduce.
"""
import sys
import types

import numpy as np

F, P, H, HEADS, C, NCLS = 18, 128, 256, 8, 32, 625
SCALES = (2, 4, 6)
HD = H // HEADS
NTOK = F * P  # 2304
NCORES = 8


def _enable_ldw_opt():
    """Walrus's LDWEIGHTS dedup is disabled by default in bass_utils;
    enable it (verified numerically by the rel-err gate in test.py)."""
    import concourse.bass_utils as bu

    if getattr(bu, "_ldw_opt_patched", False):
        return
    orig = bu.bir_verify_and_optimise

    def patched(*args, **kwargs):
        real_run = bu.run_command

        def run_hook(argv, **kw):
            argv = ["--enable-ldw-opt=true" if a == "--enable-ldw-opt=false"
                    else a for a in argv]
            return real_run(argv, **kw)

        bu.run_command = run_hook
        try:
            return orig(*args, **kwargs)
        finally:
            bu.run_command = real_run

    bu.bir_verify_and_optimise = patched
    bu._ldw_opt_patched = True


def _install_ntff_hook():
    """Recreate the missing antenv.axon_hooks so trace=True works."""
    if "antenv.axon_hooks" in sys.modules:
        return
    try:
        import antenv

        mod = types.ModuleType("antenv.axon_hooks")
        mod._hook = None
        mod.set_axon_ntff_profile_hook = lambda h: setattr(mod, "_hook", h)
        mod.get_axon_ntff_profile_hook = lambda: mod._hook
        sys.modules["antenv.axon_hooks"] = mod
        antenv.axon_hooks = mod
        from trn_agent_boot.trn_boot import _ntff_profile_via_ctypes

        mod.set_axon_ntff_profile_hook(
            _ntff_profile_via_ctypes("/opt/axon/libaxon_pjrt.so")
        )
    except Exception:
        pass


def _chunks(n, lim=512):
    out = [lim] * (n // lim)
    if n % lim:
        out.append(n % lim)
    return out


def _counts(s):
    nw = F - s + 1
    c = np.zeros(F, np.float32)
    for w in range(nw):
        c[w:w + s] += 1.0
    return c


COL_CC = _chunks(NTOK)          # [512,512,512,512,256]
AR_GROUPS = [(0, 1), (2,), (3, 4)]  # chunk-index groups per AllReduce
GROUP_FRAMES = [(0, 8), (8, 12), (12, 18)]  # frame range per AR group


def build():
    import concourse.bacc as bacc
    import concourse.mybir as mybir
    from concourse.tile import TileContext

    dt = mybir.dt
    f32 = dt.float32
    bf16 = dt.bfloat16
    f8 = dt.float8e4
    AF = mybir.ActivationFunctionType
    ALU = mybir.AluOpType
    AX = mybir.AxisListType

    nc = bacc.Bacc("TRN2", target_bir_lowering=False, debug=False,
                   num_devices=NCORES)

    # ---- I/O ----
    xt_in = nc.dram_tensor("xt", [2, 128, NTOK], bf16, kind="ExternalInput")
    wqkv_in = nc.dram_tensor("wqkv", [3, 2, 128, 65], bf16, kind="ExternalInput")
    bqkv_in = nc.dram_tensor("bqkv", [3, 65, 1], f32, kind="ExternalInput")
    wv_in = nc.dram_tensor("wv", [2, 128, 96], bf16, kind="ExternalInput")
    vb_in = nc.dram_tensor("vb", [128, 96], f32, kind="ExternalInput")
    wo_in = nc.dram_tensor("wo", [3, 32, 256], bf16, kind="ExternalInput")
    boq_in = nc.dram_tensor("boq", [2, 128, 1], f32, kind="ExternalInput")
    boqs_in = nc.dram_tensor("boqs", [2, 128, 1], f32, kind="ExternalInput")
    nhb_in = nc.dram_tensor("nhb", [18, 1], f32, kind="ExternalInput")
    clsw_in = nc.dram_tensor("clsw", [2, 128, NCLS], bf16, kind="ExternalInput")
    clsb_in = nc.dram_tensor("clsb", [1, NCLS], f32, kind="ExternalInput")
    id_in = nc.dram_tensor("ident", [128, 128], f32, kind="ExternalInput")
    out_d = nc.dram_tensor("out", [1, NCLS], f32, kind="ExternalOutput")

    ar_in_c = []
    ar_out_c = []
    for gi, grp in enumerate(AR_GROUPS):
        w = sum(COL_CC[ci] for ci in grp)
        ar_in_c.append(nc.dram_tensor(f"ar_in{gi}", [2, 128, w], bf16))
        ar_out_c.append(nc.dram_tensor(f"ar_out{gi}", [2, 128, w], bf16,
                                       addr_space="Shared"))
    ar2_in = nc.dram_tensor("ar2_in", [2, 128, 1], f32)
    ar2_out = nc.dram_tensor("ar2_out", [2, 128, 1], f32, addr_space="Shared")
    arw_in = nc.dram_tensor("arw_in", [1, 1], f32)
    arw_out = nc.dram_tensor("arw_out", [1, 1], f32, addr_space="Shared")
    hrow_d = nc.dram_tensor("hrow", [2, 1, NTOK], bf16)
    absd = nc.dram_tensor("absd", [3, 2, 1, 288], bf16)

    inv_sqrt_hd = 1.0 / np.sqrt(np.float32(HD))
    cnts = {s: _counts(s) for s in SCALES}
    grp_all = [list(range(NCORES))]

    with TileContext(nc) as tc:
        with (
            tc.tile_pool(name="pin", bufs=1) as pin,
            tc.tile_pool(name="work", bufs=3) as work,
            tc.tile_pool(name="tree", bufs=3) as tree,
            tc.tile_pool(name="estr2", bufs=4) as estr2,
            tc.tile_pool(name="estr4", bufs=8) as estr4,
            tc.tile_pool(name="estr6", bufs=12) as estr6,
            tc.tile_pool(name="ppq", bufs=2, space="PSUM") as ppq,
            tc.tile_pool(name="pstr", bufs=3, space="PSUM") as pstr,
            tc.tile_pool(name="ppT", bufs=3, space="PSUM") as ppT,
        ):
            estr = {2: estr2, 4: estr4, 6: estr6}

            # ---- persistent tiles + weight loads ----
            xt = [pin.tile([128, NTOK], bf16, tag=f"xt{c}", name=f"xt{c}")
                  for c in range(2)]
            wqkv = pin.tile([128, 3 * 2 * 65], bf16, tag="wqkv")
            bqkv = pin.tile([65, 3], f32, tag="bqkv")
            wv = pin.tile([128, 2 * 96], bf16, tag="wv")
            vb = pin.tile([128, 96], f32, tag="vb")
            wo = pin.tile([32, 3 * 256], bf16, tag="wo")
            boq = pin.tile([128, 2], f32, tag="boq")
            boqs = pin.tile([128, 2], f32, tag="boqs")
            nhb = pin.tile([18, 1], f32, tag="nhb")
            clsw = pin.tile([128, 2 * NCLS], bf16, tag="clsw")
            clsb = pin.tile([1, NCLS], f32, tag="clsb")
            ident = pin.tile([128, 128], f32, tag="ident")
            identb = pin.tile([128, 128], bf16, tag="identb")
            ones_row = pin.tile([1, 128], f32, tag="ones_row")
            ones_col = pin.tile([128, 1], f32, tag="ones_col")
            ones_colb = pin.tile([128, 1], bf16, tag="ones_colb")

            for c in range(2):
                off = 0
                for w_cc in COL_CC:
                    nc.sync.dma_start(out=xt[c][:, off:off + w_cc],
                                      in_=xt_in[c, :, off:off + w_cc])
                    off += w_cc
            for si in range(3):
                for c in range(2):
                    nc.sync.dma_start(
                        out=wqkv[:, (si * 2 + c) * 65:(si * 2 + c + 1) * 65],
                        in_=wqkv_in[si, c])
                nc.sync.dma_start(out=bqkv[:, si:si + 1], in_=bqkv_in[si])
                nc.gpsimd.dma_start(out=wo[:, si * 256:(si + 1) * 256],
                                    in_=wo_in[si])
            for c in range(2):
                nc.gpsimd.dma_start(out=wv[:, c * 96:(c + 1) * 96], in_=wv_in[c])
                nc.gpsimd.dma_start(out=boq[:, c:c + 1], in_=boq_in[c])
                nc.gpsimd.dma_start(out=boqs[:, c:c + 1], in_=boqs_in[c])
                nc.gpsimd.dma_start(out=clsw[:, c * NCLS:(c + 1) * NCLS],
                                    in_=clsw_in[c])
            nc.sync.dma_start(out=vb[:], in_=vb_in[:])
            nc.sync.dma_start(out=nhb[:], in_=nhb_in[:])
            nc.gpsimd.dma_start(out=clsb[:], in_=clsb_in[:])
            nc.gpsimd.dma_start(out=ident[:], in_=id_in[:])
            nc.vector.memset(ones_row[:], 1.0)
            nc.vector.memset(ones_col[:], 1.0)
            nc.vector.memset(ones_colb[:], 1.0)
            nc.vector.tensor_copy(identb[:], ident[:])

            qkvT = {s: pin.tile([65, NTOK], bf16, tag=f"qkvT{s}",
                                name=f"qkvT{s}") for s in SCALES}
            kT = {s: pin.tile([32, NTOK], bf16, tag=f"kT{s}",
                              name=f"kT{s}") for s in SCALES}
            # V' per frame: [V_s2|1|V_s4|1|V_s6|1] -> 99 cols, ones persist
            vp = pin.tile([128, F * 99], bf16, tag="vp")
            nc.vector.memset(vp[:], 1.0)
            abar = {s: pin.tile([128, F * 32], f32, tag=f"abar{s}",
                                name=f"abar{s}") for s in SCALES}
            abarb = {s: pin.tile([128, F * 32], bf16, tag=f"abarb{s}",
                                 name=f"abarb{s}") for s in SCALES}
            abarT = {s: pin.tile([32, NTOK], bf16, tag=f"abarT{s}",
                                 name=f"abarT{s}") for s in SCALES}
            abs32 = {s: pin.tile([32, F], bf16, tag=f"abs32{s}",
                                 name=f"abs32{s}") for s in SCALES}
            ssum0 = [pin.tile([128, F], f32, tag=f"ssum0{c}", name=f"ssum0{c}")
                     for c in range(2)]
            ssum1 = [pin.tile([128, F], f32, tag=f"ssum1{c}", name=f"ssum1{c}")
                     for c in range(2)]

            # halting state, column layout [18,1]
            ptn = pin.tile([18, 1], f32, tag="ptn")
            Rt = pin.tile([18, 1], f32, tag="Rt")
            wts = [pin.tile([18, 1], f32, tag=f"w{it}", name=f"w{it}")
                   for it in range(2)]
            nc.vector.memset(ptn[:], 0.0)
            nc.vector.memset(Rt[:], 0.0)

            # ---------------- schedule closures ----------------
            prog = {}      # (it, s) -> next kf to process
            stripmap = {}  # (it, s) -> {kf: (a_k, est_tile)}

            def qkv_chunks(it, cis):
                for si, s in enumerate(SCALES):
                    for ci in cis:
                        off = sum(COL_CC[cj] for cj in range(ci))
                        w_cc = COL_CC[ci]
                        pq = ppq.tile([65, 512], f32, tag="pq", name="pq")
                        for kc in range(2):
                            nc.tensor.matmul(
                                pq[:, :w_cc],
                                wqkv[:, (si * 2 + kc) * 65:(si * 2 + kc + 1) * 65],
                                xt[kc][:, off:off + w_cc],
                                start=(kc == 0), stop=(kc == 1))
                        nc.scalar.activation(qkvT[s][:, off:off + w_cc],
                                             pq[:, :w_cc], AF.Identity,
                                             bias=bqkv[:, si:si + 1], scale=1.0)
                        nc.sync.dma_start(out=kT[s][:, off:off + w_cc],
                                          in_=qkvT[s][32:64, off:off + w_cc])

            def vdirect(it, f0, f1):
                for f in range(f0, f1):
                    pv = ppT.tile([128, 363], f32, tag="pt", name="pv")
                    for kc in range(2):
                        nc.tensor.matmul(
                            pv[:, :96],
                            xt[kc][:, f * 128:(f + 1) * 128],
                            wv[:, kc * 96:(kc + 1) * 96],
                            start=(kc == 0), stop=(kc == 1))
                    vslc = vp[:, f * 99:(f + 1) * 99].rearrange(
                        "p (s c) -> p s c", c=33)
                    nc.vector.tensor_tensor(
                        out=vslc[:, :, 0:32],
                        in0=pv[:, :96].rearrange("p (s c) -> p s c", c=32),
                        in1=vb[:].rearrange("p (s c) -> p s c", c=32),
                        op=ALU.add)

            def halting(it):
                hl = work.tile([18, 128], bf16, tag="hl")
                nc.sync.dma_start(out=hrow_d[it], in_=qkvT[2][64:65, :])
                nc.sync.dma_start(
                    out=hl[:],
                    in_=hrow_d[it].rearrange("o (f q) -> (o f) q", q=128))
                sg = work.tile([18, 128], f32, tag="sg")
                nc.scalar.activation(sg[:], hl[:], AF.Sigmoid,
                                     bias=nhb[:], scale=1.0)
                p_t = work.tile([18, 1], f32, tag="p_t")
                nc.vector.tensor_reduce(out=p_t[:], in_=sg[:],
                                        axis=AX.X, op=ALU.add)
                nc.vector.tensor_scalar_mul(out=p_t[:], in0=p_t[:],
                                            scalar1=1.0 / 128.0)
                run_in = work.tile([18, 1], f32, tag="run_in")
                tmp = work.tile([18, 1], f32, tag="tmp")
                tmp2 = work.tile([18, 1], f32, tag="tmp2")
                nh = work.tile([18, 1], f32, tag="nh")
                run = work.tile([18, 1], f32, tag="run")
                nc.vector.tensor_scalar(out=run_in[:], in0=ptn[:], scalar1=1.0,
                                        scalar2=None, op0=ALU.is_lt)
                nc.vector.tensor_tensor(out=tmp[:], in0=p_t[:], in1=run_in[:],
                                        op=ALU.mult)
                nc.vector.tensor_tensor(out=tmp2[:], in0=ptn[:], in1=tmp[:],
                                        op=ALU.add)
                nc.vector.tensor_scalar(out=tmp2[:], in0=tmp2[:], scalar1=0.99,
                                        scalar2=None, op0=ALU.is_gt)
                nc.vector.tensor_tensor(out=nh[:], in0=tmp2[:], in1=run_in[:],
                                        op=ALU.mult)
                nc.vector.tensor_tensor(out=run[:], in0=run_in[:], in1=nh[:],
                                        op=ALU.subtract)
                nc.vector.tensor_tensor(out=tmp[:], in0=p_t[:], in1=run[:],
                                        op=ALU.mult)
                nc.vector.tensor_tensor(out=ptn[:], in0=ptn[:], in1=tmp[:],
                                        op=ALU.add)
                nc.vector.tensor_scalar(out=tmp2[:], in0=ptn[:], scalar1=-1.0,
                                        scalar2=1.0, op0=ALU.mult, op1=ALU.add)
                nc.vector.tensor_tensor(out=tmp2[:], in0=nh[:], in1=tmp2[:],
                                        op=ALU.mult)
                nc.vector.tensor_tensor(out=Rt[:], in0=Rt[:], in1=tmp2[:],
                                        op=ALU.add)
                nc.vector.tensor_tensor(out=tmp2[:], in0=nh[:], in1=Rt[:],
                                        op=ALU.mult)
                nc.vector.tensor_tensor(out=ptn[:], in0=ptn[:], in1=tmp2[:],
                                        op=ALU.add)
                nc.vector.tensor_tensor(out=wts[it][:], in0=tmp[:],
                                        in1=tmp2[:], op=ALU.add)

            def fire_group(gi):
                grp = AR_GROUPS[gi]
                f0, f1 = GROUP_FRAMES[gi]
                for sx in SCALES:
                    if sx == 6:
                        nc.scalar.copy(abarb[sx][:, f0 * 32:f1 * 32],
                                       abar[sx][:, f0 * 32:f1 * 32])
                    for f in range(f0, f1):
                        pat = ppT.tile([32, 128], bf16, tag="pt", name="pat")
                        nc.tensor.transpose(pat[:],
                                            abarb[sx][:, f * 32:(f + 1) * 32],
                                            identb[:])
                        nc.vector.tensor_copy(
                            abarT[sx][:, f * 128:(f + 1) * 128], pat[:])
                goff = sum(COL_CC[ci] for ci in range(grp[0]))
                for ci in grp:
                    off = sum(COL_CC[cj] for cj in range(ci))
                    w_cc = COL_CC[ci]
                    for hc in range(2):
                        pw = ppq.tile([128, 512], f32, tag="pq", name="pw")
                        for sj, sx in enumerate(SCALES):
                            nc.tensor.matmul(
                                pw[:, :w_cc],
                                wo[:, sj * 256 + hc * 128:sj * 256 + (hc + 1) * 128],
                                abarT[sx][:, off:off + w_cc],
                                start=(sj == 0), stop=(sj == 2))
                        nc.vector.tensor_scalar(
                            out=xt[hc][:, off:off + w_cc],
                            in0=pw[:, :w_cc],
                            scalar1=0.25, scalar2=boq[:, hc:hc + 1],
                            op0=ALU.mult, op1=ALU.add)
                        nc.sync.dma_start(
                            out=ar_in_c[gi][hc, :, off - goff:off - goff + w_cc],
                            in_=xt[hc][:, off:off + w_cc])
                nc.gpsimd.collective_compute(
                    "AllReduce", ALU.add,
                    ins=[ar_in_c[gi][:]], outs=[ar_out_c[gi][:]],
                    replica_groups=grp_all)
                gw = sum(COL_CC[ci] for ci in grp)
                for hc in range(2):
                    nc.sync.dma_start(out=xt[hc][:, goff:goff + gw],
                                      in_=ar_out_c[gi][hc])

            def advance(it, si, s, kf_limit):
                strips = stripmap.setdefault((it, s), {})
                for kf in range(prog.get((it, s), 0), kf_limit + 1):
                    a_k = max(0, kf - s + 1)
                    b_k = min(F - 1, kf + s - 1)
                    ncols = (b_k - a_k + 1) * 128
                    est_t = estr[s].tile([128, (2 * s - 1) * 128], f8,
                                         tag="est")
                    off = 0
                    for w_cc in _chunks(ncols):
                        ps = pstr.tile([128, 512], f32, tag="ps", name="ps")
                        nc.tensor.matmul(
                            ps[:, :w_cc],
                            kT[s][:, kf * 128:(kf + 1) * 128],
                            qkvT[s][0:32, a_k * 128 + off:a_k * 128 + off + w_cc],
                            start=True, stop=True)
                        nc.scalar.activation(est_t[:, off:off + w_cc],
                                             ps[:, :w_cc], AF.Exp,
                                             scale=inv_sqrt_hd)
                        off += w_cc
                    strips[kf] = (a_k, est_t)

                    if s - 1 <= kf < F - 1:
                        qlist = [kf - s + 1]
                    elif kf == F - 1:
                        qlist = list(range(F - s, F))
                    else:
                        qlist = []

                    for qi, qf in enumerate(qlist):
                        te = nc.vector
                        a = max(0, qf - s + 1)
                        b = min(F - 1, qf + s - 1)
                        W = b - a + 1
                        nwq = W - s + 1
                        pT = ppT.tile([128, 363], f32, tag="pt", name="pT")
                        for j in range(W):
                            kfj = a + j
                            ak_j, est_j = strips[kfj]
                            qo = (qf - ak_j) * 128
                            nc.tensor.matmul(
                                pT[:, j * 33:(j + 1) * 33],
                                est_j[:, qo:qo + 128],
                                vp[:, kfj * 99 + si * 33:kfj * 99 + si * 33 + 33],
                                start=True, stop=True)
                        # sliding-window sums over kf
                        Tt = tree.tile([128, 363], f32, tag="Tt")
                        nc.scalar.copy(Tt[:, :W * 33], pT[:, :W * 33])
                        U = tree.tile([128, 330], f32, tag="U")
                        te.tensor_tensor(
                            out=U[:, :(W - 1) * 33],
                            in0=Tt[:, 0:(W - 1) * 33],
                            in1=Tt[:, 33:W * 33], op=ALU.add)
                        if s == 2:
                            S_ap = U
                        else:
                            Q = tree.tile([128, 264], f32, tag="Q")
                            te.tensor_tensor(
                                out=Q[:, :(W - 3) * 33],
                                in0=U[:, 0:(W - 3) * 33],
                                in1=U[:, 66:(W - 1) * 33], op=ALU.add)
                            if s == 4:
                                S_ap = Q
                            else:
                                S6 = tree.tile([128, 198], f32, tag="S")
                                te.tensor_tensor(
                                    out=S6[:, :(W - 5) * 33],
                                    in0=Q[:, 0:(W - 5) * 33],
                                    in1=U[:, 132:(W - 1) * 33], op=ALU.add)
                                S_ap = S6
                        # normalize per window, weight by 1/count, sum
                        Sv = S_ap[:, :nwq * 33].rearrange(
                            "p (w c) -> p w c", c=33)
                        rcp = tree.tile([128, 6], f32, tag="rcp")
                        nc.vector.reciprocal(rcp[:, :nwq], Sv[:, :, 32])
                        cinv = float(1.0 / cnts[s][qf])
                        ab_out = abar[s][:, qf * 32:(qf + 1) * 32]
                        if nwq == 1:
                            nc.vector.tensor_scalar(
                                out=ab_out, in0=S_ap[:, 0:32],
                                scalar1=rcp[:, 0:1], scalar2=cinv,
                                op0=ALU.mult, op1=ALU.mult)
                        else:
                            sc = tree.tile([128, 192], f32, tag="sc")
                            scv = sc[:, :nwq * 32].rearrange(
                                "p (w c) -> p w c", c=32)
                            nc.vector.scalar_tensor_tensor(
                                out=scv, in0=Sv[:, :, 0:32], scalar=cinv,
                                in1=rcp[:, :nwq].broadcast_to((128, nwq, 32)),
                                op0=ALU.mult, op1=ALU.mult)
                            m = nwq
                            while m > 2:
                                h = m // 2
                                te.tensor_tensor(
                                    out=sc[:, (m - 2 * h) * 32:(m - h) * 32],
                                    in0=sc[:, (m - 2 * h) * 32:(m - h) * 32],
                                    in1=sc[:, (m - h) * 32:m * 32],
                                    op=ALU.add)
                                m -= h
                            te.tensor_tensor(
                                out=ab_out, in0=sc[:, 0:32],
                                in1=sc[:, 32:64], op=ALU.add)

                        if it == 0 and s == 6:
                            if qf == 7:
                                fire_group(0)
                            elif qf == 11:
                                fire_group(1)
                            elif qf == 17:
                                fire_group(2)
                        elif it == 1:
                            if qf == 8:
                                absum_half(si, s, 0)
                            elif qf == 17:
                                absum_half(si, s, 1)
                                ssum1_accum(si, s)
                prog[(it, s)] = kf_limit + 1

            def absum_half(si, s, half):
                abh = work.tile([128, 288], bf16, tag="abh")
                nc.vector.tensor_copy(abh[:],
                                      abar[s][:, half * 288:(half + 1) * 288])
                pa = ppq.tile([1, 512], f32, tag="pq", name="pa")
                nc.tensor.matmul(
                    pa[:, :288], ones_colb[:], abh[:],
                    start=True, stop=True)
                asb = work.tile([1, 288], bf16, tag="asb")
                nc.vector.tensor_copy(asb[:], pa[:, :288])
                nc.sync.dma_start(out=absd[si, half], in_=asb[0:1, :])
                nc.sync.dma_start(
                    out=abs32[s][:, half * 9:(half + 1) * 9],
                    in_=absd[si, half].rearrange(
                        "o (f c) -> (o c) f", c=32))

            def ssum1_accum(si, s):
                for hc in range(2):
                    pw1 = ppT.tile([128, 363], f32, tag="pt", name="pw1")
                    nc.tensor.matmul(
                        pw1[:, :F],
                        wo[:, si * 256 + hc * 128:si * 256 + (hc + 1) * 128],
                        abs32[s][:], start=True, stop=True)
                    if si == 0:
                        nc.vector.tensor_scalar(
                            out=ssum1[hc][:], in0=pw1[:, :F],
                            scalar1=0.25, scalar2=boqs[:, hc:hc + 1],
                            op0=ALU.mult, op1=ALU.add)
                    else:
                        nc.vector.scalar_tensor_tensor(
                            out=ssum1[hc][:], in0=pw1[:, :F], scalar=0.25,
                            in1=ssum1[hc][:], op0=ALU.mult, op1=ALU.add)

            # ---------------- schedule ----------------
            # warm-up collective: absorb inter-core launch skew while
            # weights are still loading
            wone = work.tile([1, 1], f32, tag="wone")
            nc.vector.memset(wone[:], 1.0)
            nc.sync.dma_start(out=arw_in[:], in_=wone[:])
            nc.gpsimd.collective_compute(
                "AllReduce", ALU.add, ins=[arw_in[:]], outs=[arw_out[:]],
                replica_groups=grp_all)

            qkv_chunks(0, [0, 1, 2, 3, 4])
            vdirect(0, 0, F)
            halting(0)
            for si, s in enumerate(SCALES):
                advance(0, si, s, F - 1)   # fires AR groups from inside s=6
                if s != 6:
                    nc.scalar.copy(abarb[s][:], abar[s][:])

            qkv_chunks(1, [0, 1])          # waits on AR group 0 write-back
            vdirect(1, 0, 8)
            for si, s in enumerate(SCALES):
                advance(1, si, s, 8 - s)
            qkv_chunks(1, [2])             # waits on AR group 1
            vdirect(1, 8, 12)
            for si, s in enumerate(SCALES):
                advance(1, si, s, 12 - s)
            qkv_chunks(1, [3, 4])          # waits on AR group 2
            vdirect(1, 12, F)
            halting(1)
            # coefficient row (needs wts[1] from halting(1))
            cc_t = work.tile([18, 2], f32, tag="cc_t")
            tmpc = work.tile([18, 1], f32, tag="tmpc")
            nc.vector.tensor_copy(cc_t[:, 0:1], wts[1][:])
            nc.vector.tensor_scalar(out=tmpc[:], in0=wts[1][:], scalar1=-1.0,
                                    scalar2=1.0, op0=ALU.mult, op1=ALU.add)
            nc.vector.tensor_tensor(out=tmpc[:], in0=tmpc[:], in1=wts[0][:],
                                    op=ALU.mult)
            nc.vector.tensor_scalar_mul(out=cc_t[:, 1:2], in0=tmpc[:],
                                        scalar1=1.0 / NCORES)
            ccr = work.tile([1, 36], f32, tag="ccr")
            for k in range(2):
                pcc = ppT.tile([1, 18], f32, tag="pt", name="pcc")
                nc.tensor.transpose(pcc[:], cc_t[:, k:k + 1], ident[0:18, 0:18])
                nc.vector.tensor_copy(ccr[:, k * 18:(k + 1) * 18], pcc[:])
            pc = ppT.tile([128, 36], f32, tag="pt", name="pc")
            nc.tensor.matmul(pc[:], ones_row[:], ccr[:],
                             start=True, stop=True)
            coefb = work.tile([128, 36], f32, tag="coefb")
            nc.vector.tensor_copy(coefb[:], pc[:])

            for si, s in enumerate(SCALES):
                advance(1, si, s, F - 1)   # absum/ssum1 fire from checkpoints
            for hc in range(2):
                nc.vector.tensor_reduce(
                    out=ssum0[hc][:],
                    in_=xt[hc][:].rearrange("p (f q) -> p f q", q=128),
                    axis=AX.X, op=ALU.add)

            # ============ final combine ============
            vpart = [work.tile([128, 1], f32, tag=f"vpart{hc}",
                               name=f"vpart{hc}") for hc in range(2)]
            for hc in range(2):
                t2 = work.tile([128, F], f32, tag="t2")
                nc.vector.tensor_tensor(out=t2[:], in0=ssum1[hc][:],
                                        in1=coefb[:, 0:F], op=ALU.mult)
                t1 = work.tile([128, F], f32, tag="t1")
                nc.vector.tensor_tensor(out=t1[:], in0=ssum0[hc][:],
                                        in1=coefb[:, F:2 * F], op=ALU.mult)
                nc.vector.tensor_tensor(out=t2[:], in0=t2[:], in1=t1[:],
                                        op=ALU.add)
                nc.vector.tensor_reduce(out=vpart[hc][:], in_=t2[:],
                                        axis=AX.X, op=ALU.add)
                nc.sync.dma_start(out=ar2_in[hc], in_=vpart[hc][:])
            nc.gpsimd.collective_compute(
                "AllReduce", ALU.add,
                ins=[ar2_in[:]], outs=[ar2_out[:]],
                replica_groups=grp_all)
            vfull = [work.tile([128, 1], f32, tag=f"vfull{hc}",
                               name=f"vfull{hc}") for hc in range(2)]
            vfullb = [work.tile([128, 1], bf16, tag=f"vfullb{hc}",
                                name=f"vfullb{hc}") for hc in range(2)]
            ob = work.tile([1, NCLS], f32, tag="ob")
            for hc in range(2):
                nc.sync.dma_start(out=vfull[hc][:], in_=ar2_out[hc])
                nc.vector.tensor_copy(vfullb[hc][:], vfull[hc][:])
            off = 0
            for w_cc in _chunks(NCLS):
                pcls = ppq.tile([1, 512], f32, tag="pq", name="pcls")
                for hc in range(2):
                    nc.tensor.matmul(
                        pcls[:, :w_cc], vfullb[hc][:],
                        clsw[:, hc * NCLS + off:hc * NCLS + off + w_cc],
                        start=(hc == 0), stop=(hc == 1))
                nc.vector.tensor_tensor(out=ob[:, off:off + w_cc],
                                        in0=pcls[:, :w_cc],
                                        in1=clsb[:, off:off + w_cc], op=ALU.add)
                off += w_cc
            nc.sync.dma_start(out=out_d[:], in_=ob[:])

    nc.compile()
    return nc


_NC_CACHE = None


def _get_nc():
    global _NC_CACHE
    if _NC_CACHE is None:
        _NC_CACHE = build()
    return _NC_CACHE


def _prep_in_maps(inputs):
    emb = np.ascontiguousarray(np.asarray(inputs["multiscale_embed"], np.float32))
    halt_W = np.asarray(inputs["halt_W"], np.float32)
    halt_b = np.asarray(inputs["halt_b"], np.float32)
    cls_W = np.asarray(inputs["cls_W"], np.float32)
    cls_b = np.asarray(inputs["cls_b"], np.float32)
    Wq = np.asarray(inputs["mhsa_Wq"], np.float32)
    bq = np.asarray(inputs["mhsa_bq"], np.float32)
    Wk = np.asarray(inputs["mhsa_Wk"], np.float32)
    bk = np.asarray(inputs["mhsa_bk"], np.float32)
    Wv = np.asarray(inputs["mhsa_Wv"], np.float32)
    bv = np.asarray(inputs["mhsa_bv"], np.float32)
    Wo = np.asarray(inputs["mhsa_Wo"], np.float32)
    bo = np.asarray(inputs["mhsa_bo"], np.float32)

    import ml_dtypes
    bf = ml_dtypes.bfloat16
    xt = np.ascontiguousarray(
        emb.reshape(NTOK, H).T.reshape(2, 128, NTOK)).astype(bf)
    # AllReduce sums the per-token bias over 8 cores -> divide by NCORES
    boq = np.ascontiguousarray(
        (0.25 / NCORES * bo.sum(axis=0)).reshape(2, 128, 1))
    boqs = np.ascontiguousarray(
        (32.0 / NCORES * bo.sum(axis=0)).reshape(2, 128, 1))
    hwc = halt_W.reshape(2, 128)
    nhb = np.full((18, 1), float(halt_b[0]), np.float32)
    clsw = np.ascontiguousarray(cls_W.reshape(2, 128, NCLS)).astype(bf)
    clsb = cls_b.reshape(1, NCLS).astype(np.float32)
    ident = np.eye(128, dtype=np.float32)

    in_maps = []
    for h in range(NCORES):
        sl = slice(h * HD, (h + 1) * HD)
        wqkv = np.zeros((3, 2, 128, 65), bf)
        bqkv = np.zeros((3, 65, 1), np.float32)
        wo_l = np.zeros((3, 32, 256), bf)
        wv_l = np.zeros((2, 128, 96), bf)
        vb_l = np.zeros((128, 96), np.float32)
        for si in range(3):
            blk = np.concatenate([Wq[si][:, sl], Wk[si][:, sl]], axis=1)
            wqkv[si, :, :, :64] = blk.reshape(2, 128, 64).astype(bf)
            if si == 0:
                wqkv[si, :, :, 64] = hwc.astype(bf)
            bqkv[si, :64] = np.concatenate([bq[si][sl], bk[si][sl]])[:, None]
            wo_l[si] = Wo[si][sl, :].astype(bf)
            wv_l[:, :, si * 32:(si + 1) * 32] = \
                Wv[si][:, sl].reshape(2, 128, 32).astype(bf)
            vb_l[:, si * 32:(si + 1) * 32] = bv[si][sl][None, :]
        in_maps.append({
            "xt": xt, "wqkv": wqkv, "bqkv": bqkv, "wv": wv_l, "vb": vb_l,
            "wo": wo_l, "boq": boq, "boqs": boqs, "nhb": nhb,
            "clsw": clsw, "clsb": clsb, "ident": ident,
        })
    return in_maps


def run(inputs, trace=False):
    _install_ntff_hook()
    from concourse.bass_utils import run_bass_kernel_spmd

    nc = _get_nc()
    in_maps = _prep_in_maps(inputs)
    res = run_bass_kernel_spmd(nc, in_maps, list(range(NCORES)), trace=trace)
    out = np.asarray(res.results[0]["out"], np.float32)
    return out, res


def kernel(**inputs):
    out, _ = run(inputs, trace=False)
    return out
